# revision 9
# baseline (speedup 1.0000x reference)
"""CopyNet extended-vocab projection kernel for Trainium2 (8 NeuronCores).

out[b, t, v] = p_gen[b,t] * pad(dist_t)[b,t,v] + (1 - p_gen[b,t]) * copyp[b,t,v]
copyp[b, t, v] = sum_{s: pointer[b,s]==v} alph_t[b, s, t]

Strategy: pure data-parallel over batch (B=8 -> 8 cores, one batch element per
core). All bulk I/O runs in bf16 (dist cast on host, output upcast on host,
alpha pre-scaled by (1 - p_gen) and cast on host). Per-core traffic ~33 MB;
the per-core HBM/fabric ceiling observed in traces is ~420 GB/s, so the
stream floor is ~80 us.

The copy term is a one-hot matmul on the tensor engine: onehot[s, v] =
(pointer[s] == v), built on-chip (iota + is_equal) once per 4096-wide vocab
macro-tile into a persistent SBUF arena during the prologue. Pointers are
host-grouped by owning macro-tile so each macro's matmul contracts over
K=128 grouped rows.

The PSUM drain + generation-term fuse is split across three engines so no
single engine paces the HBM stream (DVE at 1x mode needs ~2.3us per
2048-wide drain; 32 drains would exceed the stream budget):
  path_a: DVE scalar_tensor_tensor (out = dist*pgen + psum) directly.
  path_b: scalar-engine activation copies psum -> bf16 out slice, then
          GPSIMD does the in-place FMA (out = dist*pgen + out) in SBUF.
DMA dispatch cost (~0.6us per descriptor-generation instruction) is kept off
the critical path by using few, large DMAs: one combined alpha arena load,
one pgen load (host pre-layouts both), and [128, 4096] dist/out tiles.
The last macro's dist tiles are dedicated, tail-zeroed buffers so the pad
region beyond the true vocab needs no special-casing in the drain.

If any macro-tile owns more than 128 pointers (probability ~1e-9 for uniform
pointers), the kernel falls back to a dense K=512 f32 variant that makes no
assumption about pointer distribution.
"""
import sys

sys.path.insert(0, "/opt/trn_rl_repo")

import numpy as np

import concourse.bacc as bacc
import concourse.bass as bass
import concourse.tile as tile
from concourse import mybir
from concourse.bass_utils import run_bass_kernel_spmd

B = 8
L_DEC = 256
V = 32000
L_SRC = 512
V_EXT = 32128
P = 128
NCORES = 8
NPSUM = 512   # psum bank width at fp32

F32 = mybir.dt.float32
BF16 = mybir.dt.bfloat16
I16 = mybir.dt.int16
I32 = mybir.dt.int32

MACRO_SPARSE = 4096
HALF = 2048   # drain granularity (4 PSUM banks)
N_MACRO_SPARSE = (V_EXT + MACRO_SPARSE - 1) // MACRO_SPARSE  # 8 (last 3456)

_NC_CACHE = {}


def _build_nc_sparse():
    """bf16-I/O, K=128-per-macro-tile variant: host-grouped pointers."""
    nc = bacc.Bacc("TRN2", target_bir_lowering=False, debug=False)
    dist_d = nc.dram_tensor("dist", [L_DEC, V], BF16, kind="ExternalInput").ap()
    pgen_d = nc.dram_tensor("pgen2", [P, 2], F32, kind="ExternalInput").ap()
    out_d = nc.dram_tensor("out", [L_DEC, V_EXT], BF16, kind="ExternalOutput").ap()
    # agh: q-scaled alpha rows pre-grouped by owning macro-tile on the host,
    # laid out [p, m*L_DEC + t] so one DMA loads the whole arena; ptr16: the
    # grouped rows' pointer values (padded with -1), [p, m]
    agh_d = nc.dram_tensor(
        "agh", [P, N_MACRO_SPARSE * L_DEC], BF16, kind="ExternalInput"
    ).ap()
    ptr16_d = nc.dram_tensor(
        "ptr16", [P, N_MACRO_SPARSE], I16, kind="ExternalInput"
    ).ap()
    iota_d = nc.dram_tensor(
        "iota", [P, MACRO_SPARSE], I16, kind="ExternalInput"
    ).ap()

    n_tchunk = L_DEC // P
    MACRO = MACRO_SPARSE
    M_LAST = N_MACRO_SPARSE - 1
    DW_LAST = V - M_LAST * MACRO          # 3328 dist cols in last macro
    VW_LAST = V_EXT - M_LAST * MACRO      # 3456 out cols in last macro

    with tile.TileContext(nc) as tc:
        with (
            tc.tile_pool(name="const", bufs=1) as cpool,
            tc.tile_pool(name="dist", bufs=6) as dpool,
            tc.tile_pool(name="outp", bufs=4) as opool,
            tc.tile_pool(name="sh", bufs=2) as shpool,
            tc.tile_pool(name="psum", bufs=2, space="PSUM") as pspool,
        ):
            # --- prologue: one DMA per small input ---
            # iota + ptr16 + pgen ride the scalar ring (idle until the first
            # store); the alpha arena leads the sync ring ahead of the dist
            # stream
            iota16 = cpool.tile([P, MACRO], I16)
            nc.scalar.dma_start(iota16[:], iota_d[:])
            ptr16_sb = cpool.tile([P, N_MACRO_SPARSE], I16)
            nc.scalar.dma_start(ptr16_sb[:], ptr16_d[:])
            pgen_sb = cpool.tile([P, n_tchunk], F32)
            nc.scalar.dma_start(pgen_sb[:], pgen_d[:])
            agh_sb = cpool.tile([P, N_MACRO_SPARSE * L_DEC], BF16)
            nc.sync.dma_start(agh_sb[:], agh_d[:])

            # warm the ACT function table off the critical path (the first
            # ACTIVATE otherwise pays a ~1.3us lazy table load mid-pipeline)
            warm = shpool.tile([P, 1], F32, tag="warm")
            nc.scalar.activation(
                out=warm[:], in_=pgen_sb[:, 0:1],
                func=mybir.ActivationFunctionType.Copy, scale=1.0,
            )

            # last macro's dist tiles: dedicated buffers with the pad region
            # beyond the true vocab zeroed once, so every drain is a plain FMA
            dist_last = []
            for t in range(n_tchunk):
                dl = cpool.tile([P, MACRO], BF16, tag=f"dlast{t}")
                nc.gpsimd.memset(dl[:, DW_LAST:], 0.0)
                dist_last.append(dl)

            # one-hot arena, built once (DVE; TensorScalarPtr is not a legal
            # Pool-engine op so these cannot ride GPSIMD)
            oh_all = []
            for m in range(N_MACRO_SPARSE):
                vw = min(MACRO, V_EXT - m * MACRO)
                shift = shpool.tile([P, 1], F32, tag="shift")
                nc.vector.tensor_scalar(
                    out=shift[:],
                    in0=ptr16_sb[:, m : m + 1],
                    scalar1=float(m * MACRO),
                    scalar2=None, op0=mybir.AluOpType.subtract,
                )
                oh = cpool.tile([P, MACRO], BF16, tag=f"oh{m}")
                nc.vector.tensor_scalar(
                    out=oh[:, :vw], in0=iota16[:, :vw],
                    scalar1=shift[:, 0:1], scalar2=None,
                    op0=mybir.AluOpType.is_equal,
                )
                oh_all.append(oh)

            # --- main loop: one [128, 4096] tile per (macro, t-chunk) ---
            # drain pipeline per 2048-wide half: tensor engine -> ACT copies
            # PSUM into the out tile (bf16) -> DVE adds the pre-scaled dist
            # in place (tensor_tensor runs in 2x mode on all-bf16 operands).
            # dist is pre-scaled by pgen once per tile on the DVE (4x mode),
            # so the per-element FMA never runs at the DVE's 1x PSUM rate.
            for m in range(N_MACRO_SPARSE):
                v0 = m * MACRO
                vw = min(MACRO, V_EXT - v0)
                dw = max(0, min(vw, V - v0))
                oh = oh_all[m]
                for t in range(n_tchunk):
                    trow = slice(t * P, (t + 1) * P)
                    if m == M_LAST:
                        dist_sb = dist_last[t]
                        nc.sync.dma_start(
                            dist_sb[:, :dw], dist_d[trow, v0 : v0 + dw]
                        )
                    else:
                        dist_sb = dpool.tile([P, MACRO], BF16, tag="dist")
                        nc.sync.dma_start(
                            dist_sb[:], dist_d[trow, v0 : v0 + MACRO]
                        )
                    nc.vector.tensor_scalar(
                        out=dist_sb[:], in0=dist_sb[:],
                        scalar1=pgen_sb[:, t : t + 1], scalar2=None,
                        op0=mybir.AluOpType.mult,
                    )
                    out_sb = opool.tile([P, MACRO], BF16, tag="out")
                    for h in range(2):
                        c0 = h * HALF
                        hw = min(HALF, vw - c0)   # 2048, or 1408 for m=7 h=1
                        if hw <= 0:
                            continue
                        psum = pspool.tile([P, HALF], F32, space="PSUM")
                        nj = (hw + NPSUM - 1) // NPSUM
                        for j in range(nj):
                            jw = min(NPSUM, hw - j * NPSUM)
                            nc.tensor.matmul(
                                out=psum[:, j * NPSUM : j * NPSUM + jw],
                                lhsT=agh_sb[:, m * L_DEC + t * P : m * L_DEC + (t + 1) * P],
                                rhs=oh[:, c0 + j * NPSUM : c0 + j * NPSUM + jw],
                                start=True, stop=True,
                            )
                        nc.scalar.activation(
                            out=out_sb[:, c0 : c0 + hw],
                            in_=psum[:, :hw],
                            func=mybir.ActivationFunctionType.Copy,
                            scale=1.0,
                        )
                        nc.vector.tensor_tensor(
                            out=out_sb[:, c0 : c0 + hw],
                            in0=out_sb[:, c0 : c0 + hw],
                            in1=dist_sb[:, c0 : c0 + hw],
                            op=mybir.AluOpType.add,
                        )
                    # stores alternate rings so neither dispatch queue backs
                    # up; the final macro rides the (by-then idle) load ring
                    if m == M_LAST:
                        st_eng = nc.sync
                    else:
                        st_eng = nc.scalar if (m * n_tchunk + t) % 2 else nc.sync
                    st_eng.dma_start(
                        out_d[trow, v0 : v0 + vw], out_sb[:, :vw]
                    )
    nc.compile()
    return nc


def _build_nc_dense():
    """Dense K=512 f32 fallback: no assumption on pointer distribution."""
    MACRO = 2048
    nc = bacc.Bacc("TRN2", target_bir_lowering=False, debug=False)
    dist_d = nc.dram_tensor("dist", [L_DEC, V], F32, kind="ExternalInput").ap()
    pgen_d = nc.dram_tensor("pgen", [L_DEC, 1], F32, kind="ExternalInput").ap()
    alpha_d = nc.dram_tensor("alpha", [L_SRC, L_DEC], F32, kind="ExternalInput").ap()
    out_d = nc.dram_tensor("out", [L_DEC, V_EXT], F32, kind="ExternalOutput").ap()
    ptr_d = nc.dram_tensor("ptr", [L_SRC, 1], I32, kind="ExternalInput").ap()

    n_schunk = L_SRC // P
    n_tchunk = L_DEC // P
    n_macro = (V_EXT + MACRO - 1) // MACRO

    with tile.TileContext(nc) as tc:
        with (
            tc.tile_pool(name="const", bufs=1) as cpool,
            tc.tile_pool(name="dist", bufs=3) as dpool,
            tc.tile_pool(name="outp", bufs=3) as opool,
            tc.tile_pool(name="oh", bufs=2) as ohpool,
            tc.tile_pool(name="psum", bufs=6, space="PSUM") as pspool,
        ):
            ptr_sb = cpool.tile([P, n_schunk], I32)
            for c in range(n_schunk):
                nc.sync.dma_start(ptr_sb[:, c : c + 1], ptr_d[c * P : (c + 1) * P, 0:1])
            pgen_sb = cpool.tile([P, n_tchunk], F32)
            for t in range(n_tchunk):
                nc.sync.dma_start(
                    pgen_sb[:, t : t + 1], pgen_d[t * P : (t + 1) * P, 0:1]
                )
            q_sb = cpool.tile([P, n_tchunk], F32)
            nc.vector.tensor_scalar(
                out=q_sb[:], in0=pgen_sb[:], scalar1=-1.0, scalar2=1.0,
                op0=mybir.AluOpType.mult, op1=mybir.AluOpType.add,
            )
            alpha_terms = []  # per chunk: (hi, mid, lo) bf16
            for c in range(n_schunk):
                a = cpool.tile([P, L_DEC], F32, tag=f"alpha{c}")
                nc.sync.dma_start(a[:], alpha_d[c * P : (c + 1) * P, :])
                hi = cpool.tile([P, L_DEC], BF16, tag=f"ahi{c}")
                nc.vector.tensor_copy(hi[:], a[:])
                r1 = cpool.tile([P, L_DEC], F32, tag=f"r1{c}")
                nc.vector.tensor_tensor(
                    out=r1[:], in0=a[:], in1=hi[:], op=mybir.AluOpType.subtract
                )
                mid = cpool.tile([P, L_DEC], BF16, tag=f"amid{c}")
                nc.vector.tensor_copy(mid[:], r1[:])
                lo = cpool.tile([P, L_DEC], BF16, tag=f"alo{c}")
                nc.vector.tensor_tensor(
                    out=lo[:], in0=r1[:], in1=mid[:], op=mybir.AluOpType.subtract
                )
                alpha_terms.append((hi, mid, lo))
            iota16 = cpool.tile([P, MACRO], I16)
            nc.gpsimd.iota(iota16[:], pattern=[[1, MACRO]], base=0, channel_multiplier=0)

            for m in range(n_macro):
                v0 = m * MACRO
                vw = min(MACRO, V_EXT - v0)
                dw = max(0, min(vw, V - v0))
                shift = ohpool.tile([P, n_schunk], F32, tag="shift")
                nc.vector.tensor_scalar(
                    out=shift[:], in0=ptr_sb[:], scalar1=float(v0), scalar2=None,
                    op0=mybir.AluOpType.subtract,
                )
                ohs = []
                for c in range(n_schunk):
                    oh = ohpool.tile([P, MACRO], BF16, tag=f"oh{c}")
                    nc.vector.tensor_scalar(
                        out=oh[:, :vw], in0=iota16[:, :vw],
                        scalar1=shift[:, c : c + 1], scalar2=None,
                        op0=mybir.AluOpType.is_equal,
                    )
                    ohs.append(oh)
                for t in range(n_tchunk):
                    trow = slice(t * P, (t + 1) * P)
                    dist_sb = dpool.tile([P, MACRO], F32, tag="dist")
                    if dw > 0:
                        nc.sync.dma_start(dist_sb[:, :dw], dist_d[trow, v0 : v0 + dw])
                    out_sb = opool.tile([P, MACRO], F32, tag="out")
                    nj = (vw + NPSUM - 1) // NPSUM
                    for j in range(nj):
                        jw = min(NPSUM, vw - j * NPSUM)
                        psum = pspool.tile([P, NPSUM], F32, space="PSUM")
                        mm_list = [
                            (c, amat)
                            for term in range(3)
                            for c in range(n_schunk)
                            for amat in (alpha_terms[c][term],)
                        ]
                        for k, (c, amat) in enumerate(mm_list):
                            nc.tensor.matmul(
                                out=psum[:, :jw],
                                lhsT=amat[:, trow],
                                rhs=ohs[c][:, j * NPSUM : j * NPSUM + jw],
                                start=(k == 0), stop=(k == len(mm_list) - 1),
                            )
                        nc.scalar.activation(
                            out=out_sb[:, j * NPSUM : j * NPSUM + jw],
                            in_=psum[:, :jw],
                            func=mybir.ActivationFunctionType.Copy,
                            scale=q_sb[:, t : t + 1],
                        )
                    if dw > 0:
                        nc.vector.scalar_tensor_tensor(
                            out=out_sb[:, :dw], in0=dist_sb[:, :dw],
                            scalar=pgen_sb[:, t : t + 1], in1=out_sb[:, :dw],
                            op0=mybir.AluOpType.mult, op1=mybir.AluOpType.add,
                        )
                    nc.sync.dma_start(out_d[trow, v0 : v0 + vw], out_sb[:, :vw])
    nc.compile()
    return nc


def _get_nc(variant):
    if variant not in _NC_CACHE:
        _NC_CACHE[variant] = (
            _build_nc_sparse() if variant == "sparse" else _build_nc_dense()
        )
    return _NC_CACHE[variant]


_IOTA = None


def _iota_const():
    global _IOTA
    if _IOTA is None:
        _IOTA = np.ascontiguousarray(
            np.broadcast_to(
                np.arange(MACRO_SPARSE, dtype=np.int16), (P, MACRO_SPARSE)
            )
        )
    return _IOTA


def _bf16():
    import ml_dtypes

    return ml_dtypes.bfloat16


def _group_pointers(ptr_b):
    """Group source indices by owning macro-tile. Returns (idx, ptrg) each
    [N_MACRO_SPARSE, P, 1] int32, or None if any tile owns > P pointers."""
    owner = ptr_b // MACRO_SPARSE
    idx = np.zeros((N_MACRO_SPARSE, P, 1), np.int32)
    ptrg = np.full((N_MACRO_SPARSE, P, 1), -1, np.int32)
    for m in range(N_MACRO_SPARSE):
        sel = np.nonzero(owner == m)[0]
        if len(sel) > P:
            return None, None
        idx[m, : len(sel), 0] = sel
        ptrg[m, : len(sel), 0] = ptr_b[sel]
    return idx, ptrg


def _prep(dist_t, p_gen, alph_t, pointer):
    dist_t = np.asarray(dist_t, dtype=np.float32)
    p_gen = np.ascontiguousarray(
        np.asarray(p_gen, dtype=np.float32).reshape(B, L_DEC, 1)
    )
    alph_t = np.asarray(alph_t, dtype=np.float32)
    ptr = np.asarray(pointer).astype(np.int32).reshape(B, L_SRC)
    assert dist_t.shape == (B, L_DEC, V), dist_t.shape
    assert alph_t.shape == (B, L_SRC, L_DEC), alph_t.shape

    in_maps = []
    variant = "sparse"
    metas = []
    for b in range(B):
        idx, ptrg = _group_pointers(ptr[b])
        if idx is None:
            variant = "dense"
            break
        metas.append((idx, ptrg))
    if variant == "sparse":
        bf16 = _bf16()
        dist_bf = np.ascontiguousarray(dist_t.astype(bf16))
        # fold (1 - p_gen) into alpha before the bf16 round
        alphaq = (alph_t * (1.0 - p_gen.transpose(0, 2, 1))).astype(bf16)
        n_tchunk = L_DEC // P
        in_maps = []
        for b in range(B):
            idx, ptrg = metas[b]
            # gather alpha rows by owning macro on the host; zero the
            # padding rows so they contribute nothing to the matmul
            alphag = alphaq[b][idx[:, :, 0]]          # [N_MACRO, P, L_DEC]
            alphag[ptrg[:, :, 0] < 0] = 0
            # [p, m*L_DEC + t] layout -> one DMA loads the whole arena
            agh = np.ascontiguousarray(
                alphag.transpose(1, 0, 2).reshape(P, N_MACRO_SPARSE * L_DEC)
            )
            # pgen2[p, t] = p_gen[t*P + p]
            pgen2 = np.ascontiguousarray(
                p_gen[b, :, 0].reshape(n_tchunk, P).T
            )
            in_maps.append(
                {"dist": dist_bf[b], "pgen2": pgen2, "agh": agh,
                 "ptr16": np.ascontiguousarray(
                     ptrg[:, :, 0].T.astype(np.int16)),
                 "iota": _iota_const()}
            )
    else:
        dist_f32 = np.ascontiguousarray(dist_t)
        alph_f32 = np.ascontiguousarray(alph_t)
        in_maps = [
            {"dist": dist_f32[b], "pgen": p_gen[b], "alpha": alph_f32[b],
             "ptr": np.ascontiguousarray(ptr[b].reshape(L_SRC, 1))}
            for b in range(B)
        ]
    return variant, in_maps


def run(dist_t, p_gen, alph_t, batch_vocab, pointer, trace=False,
        force_variant=None, **spmd_kwargs):
    """Run the kernel; returns (output, BassKernelResults)."""
    assert batch_vocab.shape[0] == V_EXT
    variant, in_maps = _prep(dist_t, p_gen, alph_t, pointer)
    if force_variant == "dense" and variant == "sparse":
        ptr = np.asarray(pointer).astype(np.int32).reshape(B, L_SRC)
        dist_f32 = np.ascontiguousarray(np.asarray(dist_t, dtype=np.float32))
        alph_f32 = np.ascontiguousarray(np.asarray(alph_t, dtype=np.float32))
        p_gen_f = np.ascontiguousarray(
            np.asarray(p_gen, dtype=np.float32).reshape(B, L_DEC, 1)
        )
        in_maps = [
            {"dist": dist_f32[b], "pgen": p_gen_f[b], "alpha": alph_f32[b],
             "ptr": np.ascontiguousarray(ptr[b].reshape(L_SRC, 1))}
            for b in range(B)
        ]
        variant = "dense"
    run.last_variant = variant
    res = None
    for attempt in range(3):
        try:
            res = run_bass_kernel_spmd(
                _get_nc(variant), in_maps, list(range(NCORES)),
                trace=trace and attempt == 0, **spmd_kwargs
            )
            break
        except Exception:
            # transient device-state failures (e.g. NRT_EXEC_UNIT_UNRECOVERABLE
            # left over from a previous profiled session) sometimes clear on
            # retry; give it two more chances (untraced -- profiling itself
            # can be the destabilizer) before giving up
            if attempt == 2:
                raise
            import time

            time.sleep(2.0)
    outs = [res.results[b]["out"] for b in range(B)]
    out = np.stack([np.asarray(o, dtype=np.float32) for o in outs], axis=0)
    return out, res


def kernel(dist_t, p_gen, alph_t, batch_vocab, pointer):
    out, _ = run(dist_t, p_gen, alph_t, batch_vocab, pointer)
    return out


# revision 10
# speedup vs baseline: 1.0233x; 1.0233x over previous
"""CopyNet extended-vocab projection kernel for Trainium2 (8 NeuronCores).

out[b, t, v] = p_gen[b,t] * pad(dist_t)[b,t,v] + (1 - p_gen[b,t]) * copyp[b,t,v]
copyp[b, t, v] = sum_{s: pointer[b,s]==v} alph_t[b, s, t]

Strategy: pure data-parallel over batch (B=8 -> 8 cores, one batch element per
core). All bulk I/O runs in bf16 (dist cast on host, output upcast on host,
alpha pre-scaled by (1 - p_gen) and cast on host). Per-core traffic ~33 MB;
the per-core HBM/fabric ceiling observed in traces is ~420 GB/s, so the
stream floor is ~80 us.

The copy term is a one-hot matmul on the tensor engine: onehot[s, v] =
(pointer[s] == v), built on-chip (iota + is_equal) once per 4096-wide vocab
macro-tile into a persistent SBUF arena during the prologue. Pointers are
host-grouped by owning macro-tile so each macro's matmul contracts over
K=128 grouped rows.

The PSUM drain + generation-term fuse is split across three engines so no
single engine paces the HBM stream (DVE at 1x mode needs ~2.3us per
2048-wide drain; 32 drains would exceed the stream budget):
  path_a: DVE scalar_tensor_tensor (out = dist*pgen + psum) directly.
  path_b: scalar-engine activation copies psum -> bf16 out slice, then
          GPSIMD does the in-place FMA (out = dist*pgen + out) in SBUF.
DMA dispatch cost (~0.6us per descriptor-generation instruction) is kept off
the critical path by using few, large DMAs: one combined alpha arena load,
one pgen load (host pre-layouts both), and [128, 4096] dist/out tiles.
The last macro's dist tiles are dedicated, tail-zeroed buffers so the pad
region beyond the true vocab needs no special-casing in the drain.

If any macro-tile owns more than 128 pointers (probability ~1e-9 for uniform
pointers), the kernel falls back to a dense K=512 f32 variant that makes no
assumption about pointer distribution.
"""
import sys

sys.path.insert(0, "/opt/trn_rl_repo")

import numpy as np

import concourse.bacc as bacc
import concourse.bass as bass
import concourse.tile as tile
from concourse import mybir
from concourse.bass_utils import run_bass_kernel_spmd

B = 8
L_DEC = 256
V = 32000
L_SRC = 512
V_EXT = 32128
P = 128
NCORES = 8
NPSUM = 512   # psum bank width at fp32

F32 = mybir.dt.float32
BF16 = mybir.dt.bfloat16
I16 = mybir.dt.int16
I32 = mybir.dt.int32

MACRO_SPARSE = 4096
HALF = 2048   # drain granularity (4 PSUM banks)
N_MACRO_SPARSE = (V_EXT + MACRO_SPARSE - 1) // MACRO_SPARSE  # 8 (last 3456)

_NC_CACHE = {}


def _build_nc_sparse():
    """bf16-I/O, K=128-per-macro-tile variant: host-grouped pointers."""
    nc = bacc.Bacc("TRN2", target_bir_lowering=False, debug=False)
    dist_d = nc.dram_tensor("dist", [L_DEC, V], BF16, kind="ExternalInput").ap()
    pgen_d = nc.dram_tensor("pgen2", [P, 2], F32, kind="ExternalInput").ap()
    out_d = nc.dram_tensor("out", [L_DEC, V_EXT], BF16, kind="ExternalOutput").ap()
    # agh: q-scaled alpha rows pre-grouped by owning macro-tile on the host,
    # laid out [p, m*L_DEC + t] so one DMA loads the whole arena; ptr16: the
    # grouped rows' pointer values (padded with -1), [p, m]
    agh_d = nc.dram_tensor(
        "agh", [P, N_MACRO_SPARSE * L_DEC], BF16, kind="ExternalInput"
    ).ap()
    ptr16_d = nc.dram_tensor(
        "ptr16", [P, N_MACRO_SPARSE], I16, kind="ExternalInput"
    ).ap()
    iota_d = nc.dram_tensor(
        "iota", [P, MACRO_SPARSE], I16, kind="ExternalInput"
    ).ap()

    n_tchunk = L_DEC // P
    MACRO = MACRO_SPARSE
    M_LAST = N_MACRO_SPARSE - 1
    DW_LAST = V - M_LAST * MACRO          # 3328 dist cols in last macro
    VW_LAST = V_EXT - M_LAST * MACRO      # 3456 out cols in last macro

    with tile.TileContext(nc) as tc:
        with (
            tc.tile_pool(name="const", bufs=1) as cpool,
            tc.tile_pool(name="dist", bufs=6) as dpool,
            tc.tile_pool(name="outp", bufs=4) as opool,
            tc.tile_pool(name="sh", bufs=2) as shpool,
            tc.tile_pool(name="psum", bufs=2, space="PSUM") as pspool,
        ):
            # --- prologue: one DMA per small input ---
            # iota + ptr16 + pgen ride the scalar ring (idle until the first
            # store); the alpha arena leads the sync ring ahead of the dist
            # stream
            iota16 = cpool.tile([P, MACRO], I16)
            nc.scalar.dma_start(iota16[:], iota_d[:])
            ptr16_sb = cpool.tile([P, N_MACRO_SPARSE], I16)
            nc.scalar.dma_start(ptr16_sb[:], ptr16_d[:])
            pgen_sb = cpool.tile([P, n_tchunk], F32)
            nc.scalar.dma_start(pgen_sb[:], pgen_d[:])
            agh_sb = cpool.tile([P, N_MACRO_SPARSE * L_DEC], BF16)
            nc.sync.dma_start(agh_sb[:], agh_d[:])

            # warm the ACT function table off the critical path (the first
            # ACTIVATE otherwise pays a ~1.3us lazy table load mid-pipeline)
            warm = shpool.tile([P, 1], F32, tag="warm")
            nc.scalar.activation(
                out=warm[:], in_=pgen_sb[:, 0:1],
                func=mybir.ActivationFunctionType.Copy, scale=1.0,
            )

            # last macro's dist tiles: dedicated buffers with the pad region
            # beyond the true vocab zeroed once, so every drain is a plain FMA
            dist_last = []
            for t in range(n_tchunk):
                dl = cpool.tile([P, MACRO], BF16, tag=f"dlast{t}")
                nc.gpsimd.memset(dl[:, DW_LAST:], 0.0)
                dist_last.append(dl)

            # one-hot arena, built once (DVE; TensorScalarPtr is not a legal
            # Pool-engine op so these cannot ride GPSIMD)
            oh_all = []
            for m in range(N_MACRO_SPARSE):
                vw = min(MACRO, V_EXT - m * MACRO)
                shift = shpool.tile([P, 1], F32, tag="shift")
                nc.vector.tensor_scalar(
                    out=shift[:],
                    in0=ptr16_sb[:, m : m + 1],
                    scalar1=float(m * MACRO),
                    scalar2=None, op0=mybir.AluOpType.subtract,
                )
                oh = cpool.tile([P, MACRO], BF16, tag=f"oh{m}")
                nc.vector.tensor_scalar(
                    out=oh[:, :vw], in0=iota16[:, :vw],
                    scalar1=shift[:, 0:1], scalar2=None,
                    op0=mybir.AluOpType.is_equal,
                )
                oh_all.append(oh)

            # --- main loop: one [128, 4096] tile per (macro, t-chunk) ---
            # drain pipeline per 2048-wide half: tensor engine -> ACT copies
            # PSUM into the out tile (bf16) -> DVE adds the pre-scaled dist
            # in place (tensor_tensor runs in 2x mode on all-bf16 operands).
            # dist is pre-scaled by pgen once per tile on the DVE (4x mode),
            # so the per-element FMA never runs at the DVE's 1x PSUM rate.
            for m in range(N_MACRO_SPARSE):
                v0 = m * MACRO
                vw = min(MACRO, V_EXT - v0)
                dw = max(0, min(vw, V - v0))
                oh = oh_all[m]
                for t in range(n_tchunk):
                    trow = slice(t * P, (t + 1) * P)
                    if m == M_LAST:
                        dist_sb = dist_last[t]
                        nc.sync.dma_start(
                            dist_sb[:, :dw], dist_d[trow, v0 : v0 + dw]
                        )
                    else:
                        dist_sb = dpool.tile([P, MACRO], BF16, tag="dist")
                        nc.sync.dma_start(
                            dist_sb[:], dist_d[trow, v0 : v0 + MACRO]
                        )
                    nc.vector.tensor_scalar(
                        out=dist_sb[:], in0=dist_sb[:],
                        scalar1=pgen_sb[:, t : t + 1], scalar2=None,
                        op0=mybir.AluOpType.mult,
                    )
                    out_sb = opool.tile([P, MACRO], BF16, tag="out")
                    for h in range(2):
                        c0 = h * HALF
                        hw = min(HALF, vw - c0)   # 2048, or 1408 for m=7 h=1
                        if hw <= 0:
                            continue
                        psum = pspool.tile([P, HALF], F32, space="PSUM")
                        nj = (hw + NPSUM - 1) // NPSUM
                        for j in range(nj):
                            jw = min(NPSUM, hw - j * NPSUM)
                            nc.tensor.matmul(
                                out=psum[:, j * NPSUM : j * NPSUM + jw],
                                lhsT=agh_sb[:, m * L_DEC + t * P : m * L_DEC + (t + 1) * P],
                                rhs=oh[:, c0 + j * NPSUM : c0 + j * NPSUM + jw],
                                start=True, stop=True,
                            )
                        nc.scalar.activation(
                            out=out_sb[:, c0 : c0 + hw],
                            in_=psum[:, :hw],
                            func=mybir.ActivationFunctionType.Copy,
                            scale=1.0,
                        )
                        nc.vector.tensor_tensor(
                            out=out_sb[:, c0 : c0 + hw],
                            in0=out_sb[:, c0 : c0 + hw],
                            in1=dist_sb[:, c0 : c0 + hw],
                            op=mybir.AluOpType.add,
                        )
                    # stores ride the GPSIMD SWDGE queue: loads own the sync
                    # HWDGE ring and ACT drains own the scalar queue, so no
                    # store dispatch can convoy-block either (HWDGE rings are
                    # FIFO per issuing engine)
                    nc.gpsimd.dma_start(
                        out_d[trow, v0 : v0 + vw], out_sb[:, :vw]
                    )
    nc.compile()
    return nc


def _build_nc_dense():
    """Dense K=512 f32 fallback: no assumption on pointer distribution."""
    MACRO = 2048
    nc = bacc.Bacc("TRN2", target_bir_lowering=False, debug=False)
    dist_d = nc.dram_tensor("dist", [L_DEC, V], F32, kind="ExternalInput").ap()
    pgen_d = nc.dram_tensor("pgen", [L_DEC, 1], F32, kind="ExternalInput").ap()
    alpha_d = nc.dram_tensor("alpha", [L_SRC, L_DEC], F32, kind="ExternalInput").ap()
    out_d = nc.dram_tensor("out", [L_DEC, V_EXT], F32, kind="ExternalOutput").ap()
    ptr_d = nc.dram_tensor("ptr", [L_SRC, 1], I32, kind="ExternalInput").ap()

    n_schunk = L_SRC // P
    n_tchunk = L_DEC // P
    n_macro = (V_EXT + MACRO - 1) // MACRO

    with tile.TileContext(nc) as tc:
        with (
            tc.tile_pool(name="const", bufs=1) as cpool,
            tc.tile_pool(name="dist", bufs=3) as dpool,
            tc.tile_pool(name="outp", bufs=3) as opool,
            tc.tile_pool(name="oh", bufs=2) as ohpool,
            tc.tile_pool(name="psum", bufs=6, space="PSUM") as pspool,
        ):
            ptr_sb = cpool.tile([P, n_schunk], I32)
            for c in range(n_schunk):
                nc.sync.dma_start(ptr_sb[:, c : c + 1], ptr_d[c * P : (c + 1) * P, 0:1])
            pgen_sb = cpool.tile([P, n_tchunk], F32)
            for t in range(n_tchunk):
                nc.sync.dma_start(
                    pgen_sb[:, t : t + 1], pgen_d[t * P : (t + 1) * P, 0:1]
                )
            q_sb = cpool.tile([P, n_tchunk], F32)
            nc.vector.tensor_scalar(
                out=q_sb[:], in0=pgen_sb[:], scalar1=-1.0, scalar2=1.0,
                op0=mybir.AluOpType.mult, op1=mybir.AluOpType.add,
            )
            alpha_terms = []  # per chunk: (hi, mid, lo) bf16
            for c in range(n_schunk):
                a = cpool.tile([P, L_DEC], F32, tag=f"alpha{c}")
                nc.sync.dma_start(a[:], alpha_d[c * P : (c + 1) * P, :])
                hi = cpool.tile([P, L_DEC], BF16, tag=f"ahi{c}")
                nc.vector.tensor_copy(hi[:], a[:])
                r1 = cpool.tile([P, L_DEC], F32, tag=f"r1{c}")
                nc.vector.tensor_tensor(
                    out=r1[:], in0=a[:], in1=hi[:], op=mybir.AluOpType.subtract
                )
                mid = cpool.tile([P, L_DEC], BF16, tag=f"amid{c}")
                nc.vector.tensor_copy(mid[:], r1[:])
                lo = cpool.tile([P, L_DEC], BF16, tag=f"alo{c}")
                nc.vector.tensor_tensor(
                    out=lo[:], in0=r1[:], in1=mid[:], op=mybir.AluOpType.subtract
                )
                alpha_terms.append((hi, mid, lo))
            iota16 = cpool.tile([P, MACRO], I16)
            nc.gpsimd.iota(iota16[:], pattern=[[1, MACRO]], base=0, channel_multiplier=0)

            for m in range(n_macro):
                v0 = m * MACRO
                vw = min(MACRO, V_EXT - v0)
                dw = max(0, min(vw, V - v0))
                shift = ohpool.tile([P, n_schunk], F32, tag="shift")
                nc.vector.tensor_scalar(
                    out=shift[:], in0=ptr_sb[:], scalar1=float(v0), scalar2=None,
                    op0=mybir.AluOpType.subtract,
                )
                ohs = []
                for c in range(n_schunk):
                    oh = ohpool.tile([P, MACRO], BF16, tag=f"oh{c}")
                    nc.vector.tensor_scalar(
                        out=oh[:, :vw], in0=iota16[:, :vw],
                        scalar1=shift[:, c : c + 1], scalar2=None,
                        op0=mybir.AluOpType.is_equal,
                    )
                    ohs.append(oh)
                for t in range(n_tchunk):
                    trow = slice(t * P, (t + 1) * P)
                    dist_sb = dpool.tile([P, MACRO], F32, tag="dist")
                    if dw > 0:
                        nc.sync.dma_start(dist_sb[:, :dw], dist_d[trow, v0 : v0 + dw])
                    out_sb = opool.tile([P, MACRO], F32, tag="out")
                    nj = (vw + NPSUM - 1) // NPSUM
                    for j in range(nj):
                        jw = min(NPSUM, vw - j * NPSUM)
                        psum = pspool.tile([P, NPSUM], F32, space="PSUM")
                        mm_list = [
                            (c, amat)
                            for term in range(3)
                            for c in range(n_schunk)
                            for amat in (alpha_terms[c][term],)
                        ]
                        for k, (c, amat) in enumerate(mm_list):
                            nc.tensor.matmul(
                                out=psum[:, :jw],
                                lhsT=amat[:, trow],
                                rhs=ohs[c][:, j * NPSUM : j * NPSUM + jw],
                                start=(k == 0), stop=(k == len(mm_list) - 1),
                            )
                        nc.scalar.activation(
                            out=out_sb[:, j * NPSUM : j * NPSUM + jw],
                            in_=psum[:, :jw],
                            func=mybir.ActivationFunctionType.Copy,
                            scale=q_sb[:, t : t + 1],
                        )
                    if dw > 0:
                        nc.vector.scalar_tensor_tensor(
                            out=out_sb[:, :dw], in0=dist_sb[:, :dw],
                            scalar=pgen_sb[:, t : t + 1], in1=out_sb[:, :dw],
                            op0=mybir.AluOpType.mult, op1=mybir.AluOpType.add,
                        )
                    nc.sync.dma_start(out_d[trow, v0 : v0 + vw], out_sb[:, :vw])
    nc.compile()
    return nc


def _get_nc(variant):
    if variant not in _NC_CACHE:
        _NC_CACHE[variant] = (
            _build_nc_sparse() if variant == "sparse" else _build_nc_dense()
        )
    return _NC_CACHE[variant]


_IOTA = None


def _iota_const():
    global _IOTA
    if _IOTA is None:
        _IOTA = np.ascontiguousarray(
            np.broadcast_to(
                np.arange(MACRO_SPARSE, dtype=np.int16), (P, MACRO_SPARSE)
            )
        )
    return _IOTA


def _bf16():
    import ml_dtypes

    return ml_dtypes.bfloat16


def _group_pointers(ptr_b):
    """Group source indices by owning macro-tile. Returns (idx, ptrg) each
    [N_MACRO_SPARSE, P, 1] int32, or None if any tile owns > P pointers."""
    owner = ptr_b // MACRO_SPARSE
    idx = np.zeros((N_MACRO_SPARSE, P, 1), np.int32)
    ptrg = np.full((N_MACRO_SPARSE, P, 1), -1, np.int32)
    for m in range(N_MACRO_SPARSE):
        sel = np.nonzero(owner == m)[0]
        if len(sel) > P:
            return None, None
        idx[m, : len(sel), 0] = sel
        ptrg[m, : len(sel), 0] = ptr_b[sel]
    return idx, ptrg


def _prep(dist_t, p_gen, alph_t, pointer):
    dist_t = np.asarray(dist_t, dtype=np.float32)
    p_gen = np.ascontiguousarray(
        np.asarray(p_gen, dtype=np.float32).reshape(B, L_DEC, 1)
    )
    alph_t = np.asarray(alph_t, dtype=np.float32)
    ptr = np.asarray(pointer).astype(np.int32).reshape(B, L_SRC)
    assert dist_t.shape == (B, L_DEC, V), dist_t.shape
    assert alph_t.shape == (B, L_SRC, L_DEC), alph_t.shape

    in_maps = []
    variant = "sparse"
    metas = []
    for b in range(B):
        idx, ptrg = _group_pointers(ptr[b])
        if idx is None:
            variant = "dense"
            break
        metas.append((idx, ptrg))
    if variant == "sparse":
        bf16 = _bf16()
        dist_bf = np.ascontiguousarray(dist_t.astype(bf16))
        # fold (1 - p_gen) into alpha before the bf16 round
        alphaq = (alph_t * (1.0 - p_gen.transpose(0, 2, 1))).astype(bf16)
        n_tchunk = L_DEC // P
        in_maps = []
        for b in range(B):
            idx, ptrg = metas[b]
            # gather alpha rows by owning macro on the host; zero the
            # padding rows so they contribute nothing to the matmul
            alphag = alphaq[b][idx[:, :, 0]]          # [N_MACRO, P, L_DEC]
            alphag[ptrg[:, :, 0] < 0] = 0
            # [p, m*L_DEC + t] layout -> one DMA loads the whole arena
            agh = np.ascontiguousarray(
                alphag.transpose(1, 0, 2).reshape(P, N_MACRO_SPARSE * L_DEC)
            )
            # pgen2[p, t] = p_gen[t*P + p]
            pgen2 = np.ascontiguousarray(
                p_gen[b, :, 0].reshape(n_tchunk, P).T
            )
            in_maps.append(
                {"dist": dist_bf[b], "pgen2": pgen2, "agh": agh,
                 "ptr16": np.ascontiguousarray(
                     ptrg[:, :, 0].T.astype(np.int16)),
                 "iota": _iota_const()}
            )
    else:
        dist_f32 = np.ascontiguousarray(dist_t)
        alph_f32 = np.ascontiguousarray(alph_t)
        in_maps = [
            {"dist": dist_f32[b], "pgen": p_gen[b], "alpha": alph_f32[b],
             "ptr": np.ascontiguousarray(ptr[b].reshape(L_SRC, 1))}
            for b in range(B)
        ]
    return variant, in_maps


def run(dist_t, p_gen, alph_t, batch_vocab, pointer, trace=False,
        force_variant=None, **spmd_kwargs):
    """Run the kernel; returns (output, BassKernelResults)."""
    assert batch_vocab.shape[0] == V_EXT
    variant, in_maps = _prep(dist_t, p_gen, alph_t, pointer)
    if force_variant == "dense" and variant == "sparse":
        ptr = np.asarray(pointer).astype(np.int32).reshape(B, L_SRC)
        dist_f32 = np.ascontiguousarray(np.asarray(dist_t, dtype=np.float32))
        alph_f32 = np.ascontiguousarray(np.asarray(alph_t, dtype=np.float32))
        p_gen_f = np.ascontiguousarray(
            np.asarray(p_gen, dtype=np.float32).reshape(B, L_DEC, 1)
        )
        in_maps = [
            {"dist": dist_f32[b], "pgen": p_gen_f[b], "alpha": alph_f32[b],
             "ptr": np.ascontiguousarray(ptr[b].reshape(L_SRC, 1))}
            for b in range(B)
        ]
        variant = "dense"
    run.last_variant = variant
    res = None
    for attempt in range(3):
        try:
            res = run_bass_kernel_spmd(
                _get_nc(variant), in_maps, list(range(NCORES)),
                trace=trace and attempt == 0, **spmd_kwargs
            )
            break
        except Exception:
            # transient device-state failures (e.g. NRT_EXEC_UNIT_UNRECOVERABLE
            # left over from a previous profiled session) sometimes clear on
            # retry; give it two more chances (untraced -- profiling itself
            # can be the destabilizer) before giving up
            if attempt == 2:
                raise
            import time

            time.sleep(2.0)
    outs = [res.results[b]["out"] for b in range(B)]
    out = np.stack([np.asarray(o, dtype=np.float32) for o in outs], axis=0)
    return out, res


def kernel(dist_t, p_gen, alph_t, batch_vocab, pointer):
    out, _ = run(dist_t, p_gen, alph_t, batch_vocab, pointer)
    return out


# revision 14
# speedup vs baseline: 1.0620x; 1.0379x over previous
"""CopyNet extended-vocab projection kernel for Trainium2 (8 NeuronCores).

out[b, t, v] = p_gen[b,t] * pad(dist_t)[b,t,v] + (1 - p_gen[b,t]) * copyp[b,t,v]
copyp[b, t, v] = sum_{s: pointer[b,s]==v} alph_t[b, s, t]

Strategy: pure data-parallel over batch (B=8 -> 8 cores, one batch element per
core). All bulk I/O runs in bf16 (dist cast on host, output upcast on host,
alpha pre-scaled by (1 - p_gen) and cast on host). Per-core traffic ~33 MB;
the per-core HBM/fabric ceiling observed in traces is ~420 GB/s, so the
stream floor is ~80 us.

The copy term is a one-hot matmul on the tensor engine: onehot[s, v] =
(pointer[s] == v), built on-chip (iota + is_equal) once per 4096-wide vocab
macro-tile into a persistent SBUF arena during the prologue. Pointers are
host-grouped by owning macro-tile so each macro's matmul contracts over
K=128 grouped rows.

The PSUM drain + generation-term fuse is split across three engines so no
single engine paces the HBM stream (DVE at 1x mode needs ~2.3us per
2048-wide drain; 32 drains would exceed the stream budget):
  path_a: DVE scalar_tensor_tensor (out = dist*pgen + psum) directly.
  path_b: scalar-engine activation copies psum -> bf16 out slice, then
          GPSIMD does the in-place FMA (out = dist*pgen + out) in SBUF.
DMA dispatch cost (~0.6us per descriptor-generation instruction) is kept off
the critical path by using few, large DMAs: one combined alpha arena load,
one pgen load (host pre-layouts both), and [128, 4096] dist/out tiles.
The last macro's dist tiles are dedicated, tail-zeroed buffers so the pad
region beyond the true vocab needs no special-casing in the drain.

If any macro-tile owns more than 128 pointers (probability ~1e-9 for uniform
pointers), the kernel falls back to a dense K=512 f32 variant that makes no
assumption about pointer distribution.
"""
import sys

sys.path.insert(0, "/opt/trn_rl_repo")

import numpy as np

import concourse.bacc as bacc
import concourse.bass as bass
import concourse.tile as tile
from concourse import mybir
from concourse.bass_utils import run_bass_kernel_spmd

B = 8
L_DEC = 256
V = 32000
L_SRC = 512
V_EXT = 32128
P = 128
NCORES = 8
NPSUM = 512   # psum bank width at fp32

F32 = mybir.dt.float32
BF16 = mybir.dt.bfloat16
I16 = mybir.dt.int16
I32 = mybir.dt.int32

MACRO_SPARSE = 4096
HALF = 2048   # drain granularity (4 PSUM banks)
N_MACRO_SPARSE = (V_EXT + MACRO_SPARSE - 1) // MACRO_SPARSE  # 8 (last 3456)

_NC_CACHE = {}


def _build_nc_sparse():
    """bf16-I/O, K=128-per-macro-tile variant: host-grouped pointers."""
    nc = bacc.Bacc("TRN2", target_bir_lowering=False, debug=False)
    dist_d = nc.dram_tensor("dist", [L_DEC, V], BF16, kind="ExternalInput").ap()
    pgen_d = nc.dram_tensor("pgen2", [P, 2], F32, kind="ExternalInput").ap()
    out_d = nc.dram_tensor("out", [L_DEC, V_EXT], BF16, kind="ExternalOutput").ap()
    # agh: q-scaled alpha rows pre-grouped by owning macro-tile on the host,
    # laid out [p, m*L_DEC + t] so one DMA loads the whole arena; ptr16: the
    # grouped rows' pointer values (padded with -1), [p, m]
    agh_d = nc.dram_tensor(
        "agh", [P, N_MACRO_SPARSE * L_DEC], BF16, kind="ExternalInput"
    ).ap()
    ptr16_d = nc.dram_tensor(
        "ptr16", [P, N_MACRO_SPARSE], I16, kind="ExternalInput"
    ).ap()
    iota_d = nc.dram_tensor(
        "iota", [P, MACRO_SPARSE], I16, kind="ExternalInput"
    ).ap()

    n_tchunk = L_DEC // P
    MACRO = MACRO_SPARSE
    M_LAST = N_MACRO_SPARSE - 1
    DW_LAST = V - M_LAST * MACRO          # 3328 dist cols in last macro
    VW_LAST = V_EXT - M_LAST * MACRO      # 3456 out cols in last macro

    with tile.TileContext(nc) as tc:
        with (
            tc.tile_pool(name="const", bufs=1) as cpool,
            tc.tile_pool(name="dist", bufs=6) as dpool,
            tc.tile_pool(name="outp", bufs=4) as opool,
            tc.tile_pool(name="oh", bufs=2) as ohpool,
            tc.tile_pool(name="sh", bufs=2) as shpool,
            tc.tile_pool(name="psum", bufs=2, space="PSUM") as pspool,
        ):
            # --- prologue: one DMA per small input ---
            # iota leads the sync ring (HWDGE FIFO -> deterministic early
            # arrival ahead of the dist stream; it gates the first one-hot
            # and thereby the first store); ptr16 + pgen ride the scalar ring
            iota16 = cpool.tile([P, MACRO], I16)
            nc.sync.dma_start(iota16[:], iota_d[:])
            ptr16_sb = cpool.tile([P, N_MACRO_SPARSE], I16)
            nc.scalar.dma_start(ptr16_sb[:], ptr16_d[:])
            pgen_sb = cpool.tile([P, n_tchunk], F32)
            nc.scalar.dma_start(pgen_sb[:], pgen_d[:])
            agh_sb = cpool.tile([P, N_MACRO_SPARSE * L_DEC], BF16)
            nc.sync.dma_start(agh_sb[:], agh_d[:])

            # warm the ACT function table off the critical path (the first
            # ACTIVATE otherwise pays a ~1.3us lazy table load mid-pipeline)
            warm = shpool.tile([P, 1], F32, tag="warm")
            nc.scalar.activation(
                out=warm[:], in_=pgen_sb[:, 0:1],
                func=mybir.ActivationFunctionType.Copy, scale=1.0,
            )

            # last macro's dist tiles: dedicated buffers with the pad region
            # beyond the true vocab zeroed once, so every drain is a plain FMA
            dist_last = []
            for t in range(n_tchunk):
                dl = cpool.tile([P, MACRO], BF16, tag=f"dlast{t}")
                nc.gpsimd.memset(dl[:, DW_LAST:], 0.0)
                dist_last.append(dl)

            # --- main loop: one [128, 4096] tile per (macro, t-chunk) ---
            # drain pipeline per 2048-wide half: tensor engine -> ACT copies
            # PSUM into the out tile (bf16) -> DVE adds the pre-scaled dist
            # in place (tensor_tensor runs in 2x mode on all-bf16 operands).
            # dist is pre-scaled by pgen once per tile on the DVE (4x mode),
            # so the per-element FMA never runs at the DVE's 1x PSUM rate.
            for m in range(N_MACRO_SPARSE):
                v0 = m * MACRO
                vw = min(MACRO, V_EXT - v0)
                dw = max(0, min(vw, V - v0))
                # build this macro's one-hot lazily (not all upfront): the
                # Tile scheduler expresses store waits as counting-semaphore
                # thresholds over the Vector queue, so any straggling oh
                # build queued ahead of the TTs would gate EVERY store
                shift = shpool.tile([P, 1], F32, tag="shift")
                nc.vector.tensor_scalar(
                    out=shift[:],
                    in0=ptr16_sb[:, m : m + 1],
                    scalar1=float(m * MACRO),
                    scalar2=None, op0=mybir.AluOpType.subtract,
                )
                oh = ohpool.tile([P, MACRO], BF16, tag="oh")
                nc.vector.tensor_scalar(
                    out=oh[:, :vw], in0=iota16[:, :vw],
                    scalar1=shift[:, 0:1], scalar2=None,
                    op0=mybir.AluOpType.is_equal,
                )
                for t in range(n_tchunk):
                    trow = slice(t * P, (t + 1) * P)
                    if m == M_LAST:
                        dist_sb = dist_last[t]
                        nc.sync.dma_start(
                            dist_sb[:, :dw], dist_d[trow, v0 : v0 + dw]
                        )
                    else:
                        dist_sb = dpool.tile([P, MACRO], BF16, tag="dist")
                        nc.sync.dma_start(
                            dist_sb[:], dist_d[trow, v0 : v0 + MACRO]
                        )
                    nc.vector.tensor_scalar(
                        out=dist_sb[:], in0=dist_sb[:],
                        scalar1=pgen_sb[:, t : t + 1], scalar2=None,
                        op0=mybir.AluOpType.mult,
                    )
                    out_sb = opool.tile([P, MACRO], BF16, tag="out")
                    for h in range(2):
                        c0 = h * HALF
                        hw = min(HALF, vw - c0)   # 2048, or 1408 for m=7 h=1
                        if hw <= 0:
                            continue
                        psum = pspool.tile([P, HALF], F32, space="PSUM")
                        nj = (hw + NPSUM - 1) // NPSUM
                        for j in range(nj):
                            jw = min(NPSUM, hw - j * NPSUM)
                            nc.tensor.matmul(
                                out=psum[:, j * NPSUM : j * NPSUM + jw],
                                lhsT=agh_sb[:, m * L_DEC + t * P : m * L_DEC + (t + 1) * P],
                                rhs=oh[:, c0 + j * NPSUM : c0 + j * NPSUM + jw],
                                start=True, stop=True,
                            )
                        nc.scalar.activation(
                            out=out_sb[:, c0 : c0 + hw],
                            in_=psum[:, :hw],
                            func=mybir.ActivationFunctionType.Copy,
                            scale=1.0,
                        )
                        nc.vector.tensor_tensor(
                            out=out_sb[:, c0 : c0 + hw],
                            in0=out_sb[:, c0 : c0 + hw],
                            in1=dist_sb[:, c0 : c0 + hw],
                            op=mybir.AluOpType.add,
                        )
                    # stores ride the GPSIMD SWDGE queue: loads own the sync
                    # HWDGE ring and ACT drains own the scalar queue, so no
                    # store dispatch can convoy-block either (HWDGE rings are
                    # FIFO per issuing engine)
                    nc.gpsimd.dma_start(
                        out_d[trow, v0 : v0 + vw], out_sb[:, :vw]
                    )
    nc.compile()
    return nc


def _build_nc_dense():
    """Dense K=512 f32 fallback: no assumption on pointer distribution."""
    MACRO = 2048
    nc = bacc.Bacc("TRN2", target_bir_lowering=False, debug=False)
    dist_d = nc.dram_tensor("dist", [L_DEC, V], F32, kind="ExternalInput").ap()
    pgen_d = nc.dram_tensor("pgen", [L_DEC, 1], F32, kind="ExternalInput").ap()
    alpha_d = nc.dram_tensor("alpha", [L_SRC, L_DEC], F32, kind="ExternalInput").ap()
    out_d = nc.dram_tensor("out", [L_DEC, V_EXT], F32, kind="ExternalOutput").ap()
    ptr_d = nc.dram_tensor("ptr", [L_SRC, 1], I32, kind="ExternalInput").ap()

    n_schunk = L_SRC // P
    n_tchunk = L_DEC // P
    n_macro = (V_EXT + MACRO - 1) // MACRO

    with tile.TileContext(nc) as tc:
        with (
            tc.tile_pool(name="const", bufs=1) as cpool,
            tc.tile_pool(name="dist", bufs=3) as dpool,
            tc.tile_pool(name="outp", bufs=3) as opool,
            tc.tile_pool(name="oh", bufs=2) as ohpool,
            tc.tile_pool(name="psum", bufs=6, space="PSUM") as pspool,
        ):
            ptr_sb = cpool.tile([P, n_schunk], I32)
            for c in range(n_schunk):
                nc.sync.dma_start(ptr_sb[:, c : c + 1], ptr_d[c * P : (c + 1) * P, 0:1])
            pgen_sb = cpool.tile([P, n_tchunk], F32)
            for t in range(n_tchunk):
                nc.sync.dma_start(
                    pgen_sb[:, t : t + 1], pgen_d[t * P : (t + 1) * P, 0:1]
                )
            q_sb = cpool.tile([P, n_tchunk], F32)
            nc.vector.tensor_scalar(
                out=q_sb[:], in0=pgen_sb[:], scalar1=-1.0, scalar2=1.0,
                op0=mybir.AluOpType.mult, op1=mybir.AluOpType.add,
            )
            alpha_terms = []  # per chunk: (hi, mid, lo) bf16
            for c in range(n_schunk):
                a = cpool.tile([P, L_DEC], F32, tag=f"alpha{c}")
                nc.sync.dma_start(a[:], alpha_d[c * P : (c + 1) * P, :])
                hi = cpool.tile([P, L_DEC], BF16, tag=f"ahi{c}")
                nc.vector.tensor_copy(hi[:], a[:])
                r1 = cpool.tile([P, L_DEC], F32, tag=f"r1{c}")
                nc.vector.tensor_tensor(
                    out=r1[:], in0=a[:], in1=hi[:], op=mybir.AluOpType.subtract
                )
                mid = cpool.tile([P, L_DEC], BF16, tag=f"amid{c}")
                nc.vector.tensor_copy(mid[:], r1[:])
                lo = cpool.tile([P, L_DEC], BF16, tag=f"alo{c}")
                nc.vector.tensor_tensor(
                    out=lo[:], in0=r1[:], in1=mid[:], op=mybir.AluOpType.subtract
                )
                alpha_terms.append((hi, mid, lo))
            iota16 = cpool.tile([P, MACRO], I16)
            nc.gpsimd.iota(iota16[:], pattern=[[1, MACRO]], base=0, channel_multiplier=0)

            for m in range(n_macro):
                v0 = m * MACRO
                vw = min(MACRO, V_EXT - v0)
                dw = max(0, min(vw, V - v0))
                shift = ohpool.tile([P, n_schunk], F32, tag="shift")
                nc.vector.tensor_scalar(
                    out=shift[:], in0=ptr_sb[:], scalar1=float(v0), scalar2=None,
                    op0=mybir.AluOpType.subtract,
                )
                ohs = []
                for c in range(n_schunk):
                    oh = ohpool.tile([P, MACRO], BF16, tag=f"oh{c}")
                    nc.vector.tensor_scalar(
                        out=oh[:, :vw], in0=iota16[:, :vw],
                        scalar1=shift[:, c : c + 1], scalar2=None,
                        op0=mybir.AluOpType.is_equal,
                    )
                    ohs.append(oh)
                for t in range(n_tchunk):
                    trow = slice(t * P, (t + 1) * P)
                    dist_sb = dpool.tile([P, MACRO], F32, tag="dist")
                    if dw > 0:
                        nc.sync.dma_start(dist_sb[:, :dw], dist_d[trow, v0 : v0 + dw])
                    out_sb = opool.tile([P, MACRO], F32, tag="out")
                    nj = (vw + NPSUM - 1) // NPSUM
                    for j in range(nj):
                        jw = min(NPSUM, vw - j * NPSUM)
                        psum = pspool.tile([P, NPSUM], F32, space="PSUM")
                        mm_list = [
                            (c, amat)
                            for term in range(3)
                            for c in range(n_schunk)
                            for amat in (alpha_terms[c][term],)
                        ]
                        for k, (c, amat) in enumerate(mm_list):
                            nc.tensor.matmul(
                                out=psum[:, :jw],
                                lhsT=amat[:, trow],
                                rhs=ohs[c][:, j * NPSUM : j * NPSUM + jw],
                                start=(k == 0), stop=(k == len(mm_list) - 1),
                            )
                        nc.scalar.activation(
                            out=out_sb[:, j * NPSUM : j * NPSUM + jw],
                            in_=psum[:, :jw],
                            func=mybir.ActivationFunctionType.Copy,
                            scale=q_sb[:, t : t + 1],
                        )
                    if dw > 0:
                        nc.vector.scalar_tensor_tensor(
                            out=out_sb[:, :dw], in0=dist_sb[:, :dw],
                            scalar=pgen_sb[:, t : t + 1], in1=out_sb[:, :dw],
                            op0=mybir.AluOpType.mult, op1=mybir.AluOpType.add,
                        )
                    nc.sync.dma_start(out_d[trow, v0 : v0 + vw], out_sb[:, :vw])
    nc.compile()
    return nc


def _get_nc(variant):
    if variant not in _NC_CACHE:
        _NC_CACHE[variant] = (
            _build_nc_sparse() if variant == "sparse" else _build_nc_dense()
        )
    return _NC_CACHE[variant]


_IOTA = None


def _iota_const():
    global _IOTA
    if _IOTA is None:
        _IOTA = np.ascontiguousarray(
            np.broadcast_to(
                np.arange(MACRO_SPARSE, dtype=np.int16), (P, MACRO_SPARSE)
            )
        )
    return _IOTA


def _bf16():
    import ml_dtypes

    return ml_dtypes.bfloat16


def _group_pointers(ptr_b):
    """Group source indices by owning macro-tile. Returns (idx, ptrg) each
    [N_MACRO_SPARSE, P, 1] int32, or None if any tile owns > P pointers."""
    owner = ptr_b // MACRO_SPARSE
    idx = np.zeros((N_MACRO_SPARSE, P, 1), np.int32)
    ptrg = np.full((N_MACRO_SPARSE, P, 1), -1, np.int32)
    for m in range(N_MACRO_SPARSE):
        sel = np.nonzero(owner == m)[0]
        if len(sel) > P:
            return None, None
        idx[m, : len(sel), 0] = sel
        ptrg[m, : len(sel), 0] = ptr_b[sel]
    return idx, ptrg


def _prep(dist_t, p_gen, alph_t, pointer):
    dist_t = np.asarray(dist_t, dtype=np.float32)
    p_gen = np.ascontiguousarray(
        np.asarray(p_gen, dtype=np.float32).reshape(B, L_DEC, 1)
    )
    alph_t = np.asarray(alph_t, dtype=np.float32)
    ptr = np.asarray(pointer).astype(np.int32).reshape(B, L_SRC)
    assert dist_t.shape == (B, L_DEC, V), dist_t.shape
    assert alph_t.shape == (B, L_SRC, L_DEC), alph_t.shape

    in_maps = []
    variant = "sparse"
    metas = []
    for b in range(B):
        idx, ptrg = _group_pointers(ptr[b])
        if idx is None:
            variant = "dense"
            break
        metas.append((idx, ptrg))
    if variant == "sparse":
        bf16 = _bf16()
        dist_bf = np.ascontiguousarray(dist_t.astype(bf16))
        # fold (1 - p_gen) into alpha before the bf16 round
        alphaq = (alph_t * (1.0 - p_gen.transpose(0, 2, 1))).astype(bf16)
        n_tchunk = L_DEC // P
        in_maps = []
        for b in range(B):
            idx, ptrg = metas[b]
            # gather alpha rows by owning macro on the host; zero the
            # padding rows so they contribute nothing to the matmul
            alphag = alphaq[b][idx[:, :, 0]]          # [N_MACRO, P, L_DEC]
            alphag[ptrg[:, :, 0] < 0] = 0
            # [p, m*L_DEC + t] layout -> one DMA loads the whole arena
            agh = np.ascontiguousarray(
                alphag.transpose(1, 0, 2).reshape(P, N_MACRO_SPARSE * L_DEC)
            )
            # pgen2[p, t] = p_gen[t*P + p]
            pgen2 = np.ascontiguousarray(
                p_gen[b, :, 0].reshape(n_tchunk, P).T
            )
            in_maps.append(
                {"dist": dist_bf[b], "pgen2": pgen2, "agh": agh,
                 "ptr16": np.ascontiguousarray(
                     ptrg[:, :, 0].T.astype(np.int16)),
                 "iota": _iota_const()}
            )
    else:
        dist_f32 = np.ascontiguousarray(dist_t)
        alph_f32 = np.ascontiguousarray(alph_t)
        in_maps = [
            {"dist": dist_f32[b], "pgen": p_gen[b], "alpha": alph_f32[b],
             "ptr": np.ascontiguousarray(ptr[b].reshape(L_SRC, 1))}
            for b in range(B)
        ]
    return variant, in_maps


def run(dist_t, p_gen, alph_t, batch_vocab, pointer, trace=False,
        force_variant=None, **spmd_kwargs):
    """Run the kernel; returns (output, BassKernelResults)."""
    assert batch_vocab.shape[0] == V_EXT
    variant, in_maps = _prep(dist_t, p_gen, alph_t, pointer)
    if force_variant == "dense" and variant == "sparse":
        ptr = np.asarray(pointer).astype(np.int32).reshape(B, L_SRC)
        dist_f32 = np.ascontiguousarray(np.asarray(dist_t, dtype=np.float32))
        alph_f32 = np.ascontiguousarray(np.asarray(alph_t, dtype=np.float32))
        p_gen_f = np.ascontiguousarray(
            np.asarray(p_gen, dtype=np.float32).reshape(B, L_DEC, 1)
        )
        in_maps = [
            {"dist": dist_f32[b], "pgen": p_gen_f[b], "alpha": alph_f32[b],
             "ptr": np.ascontiguousarray(ptr[b].reshape(L_SRC, 1))}
            for b in range(B)
        ]
        variant = "dense"
    run.last_variant = variant
    res = None
    for attempt in range(3):
        try:
            res = run_bass_kernel_spmd(
                _get_nc(variant), in_maps, list(range(NCORES)),
                trace=trace and attempt == 0, **spmd_kwargs
            )
            break
        except Exception:
            # transient device-state failures (e.g. NRT_EXEC_UNIT_UNRECOVERABLE
            # left over from a previous profiled session) sometimes clear on
            # retry; give it two more chances (untraced -- profiling itself
            # can be the destabilizer) before giving up
            if attempt == 2:
                raise
            import time

            time.sleep(2.0)
    outs = [res.results[b]["out"] for b in range(B)]
    out = np.stack([np.asarray(o, dtype=np.float32) for o in outs], axis=0)
    return out, res


def kernel(dist_t, p_gen, alph_t, batch_vocab, pointer):
    out, _ = run(dist_t, p_gen, alph_t, batch_vocab, pointer)
    return out


# revision 16
# speedup vs baseline: 1.0991x; 1.0349x over previous
"""CopyNet extended-vocab projection kernel for Trainium2 (8 NeuronCores).

out[b, t, v] = p_gen[b,t] * pad(dist_t)[b,t,v] + (1 - p_gen[b,t]) * copyp[b,t,v]
copyp[b, t, v] = sum_{s: pointer[b,s]==v} alph_t[b, s, t]

Strategy: pure data-parallel over batch (B=8 -> 8 cores, one batch element per
core). All bulk I/O runs in bf16 (dist cast on host, output upcast on host,
alpha pre-scaled by (1 - p_gen) and cast on host). Per-core traffic ~33 MB;
the per-core HBM/fabric ceiling observed in traces is ~420 GB/s, so the
stream floor is ~80 us.

The copy term is a one-hot matmul on the tensor engine: onehot[s, v] =
(pointer[s] == v), built on-chip (iota + is_equal) once per 4096-wide vocab
macro-tile into a persistent SBUF arena during the prologue. Pointers are
host-grouped by owning macro-tile so each macro's matmul contracts over
K=128 grouped rows.

The PSUM drain + generation-term fuse is split across three engines so no
single engine paces the HBM stream (DVE at 1x mode needs ~2.3us per
2048-wide drain; 32 drains would exceed the stream budget):
  path_a: DVE scalar_tensor_tensor (out = dist*pgen + psum) directly.
  path_b: scalar-engine activation copies psum -> bf16 out slice, then
          GPSIMD does the in-place FMA (out = dist*pgen + out) in SBUF.
DMA dispatch cost (~0.6us per descriptor-generation instruction) is kept off
the critical path by using few, large DMAs: one combined alpha arena load,
one pgen load (host pre-layouts both), and [128, 4096] dist/out tiles.
The last macro's dist tiles are dedicated, tail-zeroed buffers so the pad
region beyond the true vocab needs no special-casing in the drain.

If any macro-tile owns more than 128 pointers (probability ~1e-9 for uniform
pointers), the kernel falls back to a dense K=512 f32 variant that makes no
assumption about pointer distribution.
"""
import sys

sys.path.insert(0, "/opt/trn_rl_repo")

import numpy as np

import concourse.bacc as bacc
import concourse.bass as bass
import concourse.tile as tile
from concourse import mybir
from concourse.bass_utils import run_bass_kernel_spmd

B = 8
L_DEC = 256
V = 32000
L_SRC = 512
V_EXT = 32128
P = 128
NCORES = 8
NPSUM = 512   # psum bank width at fp32

F32 = mybir.dt.float32
BF16 = mybir.dt.bfloat16
I16 = mybir.dt.int16
I32 = mybir.dt.int32

MACRO_SPARSE = 4096
HALF = 2048   # drain granularity (4 PSUM banks)
N_MACRO_SPARSE = (V_EXT + MACRO_SPARSE - 1) // MACRO_SPARSE  # 8 (last 3456)

_NC_CACHE = {}


def _build_nc_sparse():
    """bf16-I/O, K=128-per-macro-tile variant: host-grouped pointers."""
    nc = bacc.Bacc("TRN2", target_bir_lowering=False, debug=False)
    dist_d = nc.dram_tensor("dist", [L_DEC, V], BF16, kind="ExternalInput").ap()
    out_d = nc.dram_tensor("out", [L_DEC, V_EXT], BF16, kind="ExternalOutput").ap()
    # agh: q-scaled alpha rows pre-grouped by owning macro-tile on the host,
    # laid out [p, m*L_DEC + t] so one DMA loads the whole arena
    agh_d = nc.dram_tensor(
        "agh", [P, N_MACRO_SPARSE * L_DEC], BF16, kind="ExternalInput"
    ).ap()
    # smalls packs pgen (cols 0..1) and the grouped pointer values as f32
    # (cols 2..9), padded to 128 cols: 512 B per partition keeps the DMA
    # descriptors at line rate (8/16-byte descriptors take 7-12us to land
    # under early HBM contention and gated the whole drain pipeline)
    smalls_d = nc.dram_tensor(
        "smalls", [P, P], F32, kind="ExternalInput"
    ).ap()
    iota_d = nc.dram_tensor(
        "iota", [P, MACRO_SPARSE], I16, kind="ExternalInput"
    ).ap()

    n_tchunk = L_DEC // P
    MACRO = MACRO_SPARSE
    M_LAST = N_MACRO_SPARSE - 1
    DW_LAST = V - M_LAST * MACRO          # 3328 dist cols in last macro
    VW_LAST = V_EXT - M_LAST * MACRO      # 3456 out cols in last macro

    with tile.TileContext(nc) as tc:
        with (
            tc.tile_pool(name="const", bufs=1) as cpool,
            tc.tile_pool(name="dist", bufs=6) as dpool,
            tc.tile_pool(name="outp", bufs=4) as opool,
            tc.tile_pool(name="oh", bufs=2) as ohpool,
            tc.tile_pool(name="sh", bufs=2) as shpool,
            tc.tile_pool(name="psum", bufs=2, space="PSUM") as pspool,
        ):
            # --- prologue: one DMA per small input ---
            # iota leads the sync ring (HWDGE FIFO -> deterministic early
            # arrival ahead of the dist stream; it gates the first one-hot
            # and thereby the first store); ptr16 + pgen ride the scalar ring
            iota16 = cpool.tile([P, MACRO], I16)
            nc.sync.dma_start(iota16[:], iota_d[:])
            smalls_sb = cpool.tile([P, P], F32)
            nc.scalar.dma_start(smalls_sb[:], smalls_d[:])
            pgen_sb = smalls_sb[:, 0:n_tchunk]
            ptr_sb = smalls_sb[:, n_tchunk : n_tchunk + N_MACRO_SPARSE]
            agh_sb = cpool.tile([P, N_MACRO_SPARSE * L_DEC], BF16)
            nc.sync.dma_start(agh_sb[:], agh_d[:])

            # warm the ACT function table off the critical path (the first
            # ACTIVATE otherwise pays a ~1.3us lazy table load mid-pipeline)
            warm = shpool.tile([P, 1], F32, tag="warm")
            nc.scalar.activation(
                out=warm[:], in_=smalls_sb[:, 0:1],
                func=mybir.ActivationFunctionType.Copy, scale=1.0,
            )

            # last macro's dist tiles: dedicated buffers with the pad region
            # beyond the true vocab zeroed once, so every drain is a plain FMA
            dist_last = []
            for t in range(n_tchunk):
                dl = cpool.tile([P, MACRO], BF16, tag=f"dlast{t}")
                nc.gpsimd.memset(dl[:, DW_LAST:], 0.0)
                dist_last.append(dl)

            # --- main loop: one [128, 4096] tile per (macro, t-chunk) ---
            # drain pipeline per 2048-wide half: tensor engine -> ACT copies
            # PSUM into the out tile (bf16) -> DVE adds the pre-scaled dist
            # in place (tensor_tensor runs in 2x mode on all-bf16 operands).
            # dist is pre-scaled by pgen once per tile on the DVE (4x mode),
            # so the per-element FMA never runs at the DVE's 1x PSUM rate.
            for m in range(N_MACRO_SPARSE):
                v0 = m * MACRO
                vw = min(MACRO, V_EXT - v0)
                dw = max(0, min(vw, V - v0))
                # build this macro's one-hot lazily (not all upfront): the
                # Tile scheduler expresses store waits as counting-semaphore
                # thresholds over the Vector queue, so any straggling oh
                # build queued ahead of the TTs would gate EVERY store
                shift = shpool.tile([P, 1], F32, tag="shift")
                nc.vector.tensor_scalar(
                    out=shift[:],
                    in0=smalls_sb[:, n_tchunk + m : n_tchunk + m + 1],
                    scalar1=float(m * MACRO),
                    scalar2=None, op0=mybir.AluOpType.subtract,
                )
                oh = ohpool.tile([P, MACRO], BF16, tag="oh")
                nc.vector.tensor_scalar(
                    out=oh[:, :vw], in0=iota16[:, :vw],
                    scalar1=shift[:, 0:1], scalar2=None,
                    op0=mybir.AluOpType.is_equal,
                )
                for t in range(n_tchunk):
                    trow = slice(t * P, (t + 1) * P)
                    if m == M_LAST:
                        dist_sb = dist_last[t]
                        nc.sync.dma_start(
                            dist_sb[:, :dw], dist_d[trow, v0 : v0 + dw]
                        )
                    else:
                        dist_sb = dpool.tile([P, MACRO], BF16, tag="dist")
                        nc.sync.dma_start(
                            dist_sb[:], dist_d[trow, v0 : v0 + MACRO]
                        )
                    nc.vector.tensor_scalar(
                        out=dist_sb[:], in0=dist_sb[:],
                        scalar1=smalls_sb[:, t : t + 1], scalar2=None,
                        op0=mybir.AluOpType.mult,
                    )
                    out_sb = opool.tile([P, MACRO], BF16, tag="out")
                    for h in range(2):
                        c0 = h * HALF
                        hw = min(HALF, vw - c0)   # 2048, or 1408 for m=7 h=1
                        if hw <= 0:
                            continue
                        psum = pspool.tile([P, HALF], F32, space="PSUM")
                        nj = (hw + NPSUM - 1) // NPSUM
                        for j in range(nj):
                            jw = min(NPSUM, hw - j * NPSUM)
                            nc.tensor.matmul(
                                out=psum[:, j * NPSUM : j * NPSUM + jw],
                                lhsT=agh_sb[:, m * L_DEC + t * P : m * L_DEC + (t + 1) * P],
                                rhs=oh[:, c0 + j * NPSUM : c0 + j * NPSUM + jw],
                                start=True, stop=True,
                            )
                        nc.scalar.activation(
                            out=out_sb[:, c0 : c0 + hw],
                            in_=psum[:, :hw],
                            func=mybir.ActivationFunctionType.Copy,
                            scale=1.0,
                        )
                        nc.vector.tensor_tensor(
                            out=out_sb[:, c0 : c0 + hw],
                            in0=out_sb[:, c0 : c0 + hw],
                            in1=dist_sb[:, c0 : c0 + hw],
                            op=mybir.AluOpType.add,
                        )
                    # stores ride the GPSIMD SWDGE queue: loads own the sync
                    # HWDGE ring and ACT drains own the scalar queue, so no
                    # store dispatch can convoy-block either (HWDGE rings are
                    # FIFO per issuing engine)
                    nc.gpsimd.dma_start(
                        out_d[trow, v0 : v0 + vw], out_sb[:, :vw]
                    )
    nc.compile()
    return nc


def _build_nc_dense():
    """Dense K=512 f32 fallback: no assumption on pointer distribution."""
    MACRO = 2048
    nc = bacc.Bacc("TRN2", target_bir_lowering=False, debug=False)
    dist_d = nc.dram_tensor("dist", [L_DEC, V], F32, kind="ExternalInput").ap()
    pgen_d = nc.dram_tensor("pgen", [L_DEC, 1], F32, kind="ExternalInput").ap()
    alpha_d = nc.dram_tensor("alpha", [L_SRC, L_DEC], F32, kind="ExternalInput").ap()
    out_d = nc.dram_tensor("out", [L_DEC, V_EXT], F32, kind="ExternalOutput").ap()
    ptr_d = nc.dram_tensor("ptr", [L_SRC, 1], I32, kind="ExternalInput").ap()

    n_schunk = L_SRC // P
    n_tchunk = L_DEC // P
    n_macro = (V_EXT + MACRO - 1) // MACRO

    with tile.TileContext(nc) as tc:
        with (
            tc.tile_pool(name="const", bufs=1) as cpool,
            tc.tile_pool(name="dist", bufs=3) as dpool,
            tc.tile_pool(name="outp", bufs=3) as opool,
            tc.tile_pool(name="oh", bufs=2) as ohpool,
            tc.tile_pool(name="psum", bufs=6, space="PSUM") as pspool,
        ):
            ptr_sb = cpool.tile([P, n_schunk], I32)
            for c in range(n_schunk):
                nc.sync.dma_start(ptr_sb[:, c : c + 1], ptr_d[c * P : (c + 1) * P, 0:1])
            pgen_sb = cpool.tile([P, n_tchunk], F32)
            for t in range(n_tchunk):
                nc.sync.dma_start(
                    pgen_sb[:, t : t + 1], pgen_d[t * P : (t + 1) * P, 0:1]
                )
            q_sb = cpool.tile([P, n_tchunk], F32)
            nc.vector.tensor_scalar(
                out=q_sb[:], in0=pgen_sb[:], scalar1=-1.0, scalar2=1.0,
                op0=mybir.AluOpType.mult, op1=mybir.AluOpType.add,
            )
            alpha_terms = []  # per chunk: (hi, mid, lo) bf16
            for c in range(n_schunk):
                a = cpool.tile([P, L_DEC], F32, tag=f"alpha{c}")
                nc.sync.dma_start(a[:], alpha_d[c * P : (c + 1) * P, :])
                hi = cpool.tile([P, L_DEC], BF16, tag=f"ahi{c}")
                nc.vector.tensor_copy(hi[:], a[:])
                r1 = cpool.tile([P, L_DEC], F32, tag=f"r1{c}")
                nc.vector.tensor_tensor(
                    out=r1[:], in0=a[:], in1=hi[:], op=mybir.AluOpType.subtract
                )
                mid = cpool.tile([P, L_DEC], BF16, tag=f"amid{c}")
                nc.vector.tensor_copy(mid[:], r1[:])
                lo = cpool.tile([P, L_DEC], BF16, tag=f"alo{c}")
                nc.vector.tensor_tensor(
                    out=lo[:], in0=r1[:], in1=mid[:], op=mybir.AluOpType.subtract
                )
                alpha_terms.append((hi, mid, lo))
            iota16 = cpool.tile([P, MACRO], I16)
            nc.gpsimd.iota(iota16[:], pattern=[[1, MACRO]], base=0, channel_multiplier=0)

            for m in range(n_macro):
                v0 = m * MACRO
                vw = min(MACRO, V_EXT - v0)
                dw = max(0, min(vw, V - v0))
                shift = ohpool.tile([P, n_schunk], F32, tag="shift")
                nc.vector.tensor_scalar(
                    out=shift[:], in0=ptr_sb[:], scalar1=float(v0), scalar2=None,
                    op0=mybir.AluOpType.subtract,
                )
                ohs = []
                for c in range(n_schunk):
                    oh = ohpool.tile([P, MACRO], BF16, tag=f"oh{c}")
                    nc.vector.tensor_scalar(
                        out=oh[:, :vw], in0=iota16[:, :vw],
                        scalar1=shift[:, c : c + 1], scalar2=None,
                        op0=mybir.AluOpType.is_equal,
                    )
                    ohs.append(oh)
                for t in range(n_tchunk):
                    trow = slice(t * P, (t + 1) * P)
                    dist_sb = dpool.tile([P, MACRO], F32, tag="dist")
                    if dw > 0:
                        nc.sync.dma_start(dist_sb[:, :dw], dist_d[trow, v0 : v0 + dw])
                    out_sb = opool.tile([P, MACRO], F32, tag="out")
                    nj = (vw + NPSUM - 1) // NPSUM
                    for j in range(nj):
                        jw = min(NPSUM, vw - j * NPSUM)
                        psum = pspool.tile([P, NPSUM], F32, space="PSUM")
                        mm_list = [
                            (c, amat)
                            for term in range(3)
                            for c in range(n_schunk)
                            for amat in (alpha_terms[c][term],)
                        ]
                        for k, (c, amat) in enumerate(mm_list):
                            nc.tensor.matmul(
                                out=psum[:, :jw],
                                lhsT=amat[:, trow],
                                rhs=ohs[c][:, j * NPSUM : j * NPSUM + jw],
                                start=(k == 0), stop=(k == len(mm_list) - 1),
                            )
                        nc.scalar.activation(
                            out=out_sb[:, j * NPSUM : j * NPSUM + jw],
                            in_=psum[:, :jw],
                            func=mybir.ActivationFunctionType.Copy,
                            scale=q_sb[:, t : t + 1],
                        )
                    if dw > 0:
                        nc.vector.scalar_tensor_tensor(
                            out=out_sb[:, :dw], in0=dist_sb[:, :dw],
                            scalar=pgen_sb[:, t : t + 1], in1=out_sb[:, :dw],
                            op0=mybir.AluOpType.mult, op1=mybir.AluOpType.add,
                        )
                    nc.sync.dma_start(out_d[trow, v0 : v0 + vw], out_sb[:, :vw])
    nc.compile()
    return nc


def _get_nc(variant):
    if variant not in _NC_CACHE:
        _NC_CACHE[variant] = (
            _build_nc_sparse() if variant == "sparse" else _build_nc_dense()
        )
    return _NC_CACHE[variant]


_IOTA = None


def _iota_const():
    global _IOTA
    if _IOTA is None:
        _IOTA = np.ascontiguousarray(
            np.broadcast_to(
                np.arange(MACRO_SPARSE, dtype=np.int16), (P, MACRO_SPARSE)
            )
        )
    return _IOTA


def _bf16():
    import ml_dtypes

    return ml_dtypes.bfloat16


def _group_pointers(ptr_b):
    """Group source indices by owning macro-tile. Returns (idx, ptrg) each
    [N_MACRO_SPARSE, P, 1] int32, or None if any tile owns > P pointers."""
    owner = ptr_b // MACRO_SPARSE
    idx = np.zeros((N_MACRO_SPARSE, P, 1), np.int32)
    ptrg = np.full((N_MACRO_SPARSE, P, 1), -1, np.int32)
    for m in range(N_MACRO_SPARSE):
        sel = np.nonzero(owner == m)[0]
        if len(sel) > P:
            return None, None
        idx[m, : len(sel), 0] = sel
        ptrg[m, : len(sel), 0] = ptr_b[sel]
    return idx, ptrg


def _prep(dist_t, p_gen, alph_t, pointer):
    dist_t = np.asarray(dist_t, dtype=np.float32)
    p_gen = np.ascontiguousarray(
        np.asarray(p_gen, dtype=np.float32).reshape(B, L_DEC, 1)
    )
    alph_t = np.asarray(alph_t, dtype=np.float32)
    ptr = np.asarray(pointer).astype(np.int32).reshape(B, L_SRC)
    assert dist_t.shape == (B, L_DEC, V), dist_t.shape
    assert alph_t.shape == (B, L_SRC, L_DEC), alph_t.shape

    in_maps = []
    variant = "sparse"
    metas = []
    for b in range(B):
        idx, ptrg = _group_pointers(ptr[b])
        if idx is None:
            variant = "dense"
            break
        metas.append((idx, ptrg))
    if variant == "sparse":
        bf16 = _bf16()
        dist_bf = np.ascontiguousarray(dist_t.astype(bf16))
        # fold (1 - p_gen) into alpha before the bf16 round
        alphaq = (alph_t * (1.0 - p_gen.transpose(0, 2, 1))).astype(bf16)
        n_tchunk = L_DEC // P
        in_maps = []
        for b in range(B):
            idx, ptrg = metas[b]
            # gather alpha rows by owning macro on the host; zero the
            # padding rows so they contribute nothing to the matmul
            alphag = alphaq[b][idx[:, :, 0]]          # [N_MACRO, P, L_DEC]
            alphag[ptrg[:, :, 0] < 0] = 0
            # [p, m*L_DEC + t] layout -> one DMA loads the whole arena
            agh = np.ascontiguousarray(
                alphag.transpose(1, 0, 2).reshape(P, N_MACRO_SPARSE * L_DEC)
            )
            # smalls[p, 0:2] = pgen per t-chunk, smalls[p, 2:10] = grouped
            # pointer values as f32, padded to 128 cols for line-rate DMA
            smalls = np.zeros((P, P), np.float32)
            smalls[:, :n_tchunk] = p_gen[b, :, 0].reshape(n_tchunk, P).T
            smalls[:, n_tchunk : n_tchunk + N_MACRO_SPARSE] = (
                ptrg[:, :, 0].T.astype(np.float32)
            )
            in_maps.append(
                {"dist": dist_bf[b], "smalls": smalls, "agh": agh,
                 "iota": _iota_const()}
            )
    else:
        dist_f32 = np.ascontiguousarray(dist_t)
        alph_f32 = np.ascontiguousarray(alph_t)
        in_maps = [
            {"dist": dist_f32[b], "pgen": p_gen[b], "alpha": alph_f32[b],
             "ptr": np.ascontiguousarray(ptr[b].reshape(L_SRC, 1))}
            for b in range(B)
        ]
    return variant, in_maps


def run(dist_t, p_gen, alph_t, batch_vocab, pointer, trace=False,
        force_variant=None, **spmd_kwargs):
    """Run the kernel; returns (output, BassKernelResults)."""
    assert batch_vocab.shape[0] == V_EXT
    variant, in_maps = _prep(dist_t, p_gen, alph_t, pointer)
    if force_variant == "dense" and variant == "sparse":
        ptr = np.asarray(pointer).astype(np.int32).reshape(B, L_SRC)
        dist_f32 = np.ascontiguousarray(np.asarray(dist_t, dtype=np.float32))
        alph_f32 = np.ascontiguousarray(np.asarray(alph_t, dtype=np.float32))
        p_gen_f = np.ascontiguousarray(
            np.asarray(p_gen, dtype=np.float32).reshape(B, L_DEC, 1)
        )
        in_maps = [
            {"dist": dist_f32[b], "pgen": p_gen_f[b], "alpha": alph_f32[b],
             "ptr": np.ascontiguousarray(ptr[b].reshape(L_SRC, 1))}
            for b in range(B)
        ]
        variant = "dense"
    run.last_variant = variant
    res = None
    for attempt in range(3):
        try:
            res = run_bass_kernel_spmd(
                _get_nc(variant), in_maps, list(range(NCORES)),
                trace=trace and attempt == 0, **spmd_kwargs
            )
            break
        except Exception:
            # transient device-state failures (e.g. NRT_EXEC_UNIT_UNRECOVERABLE
            # left over from a previous profiled session) sometimes clear on
            # retry; give it two more chances (untraced -- profiling itself
            # can be the destabilizer) before giving up
            if attempt == 2:
                raise
            import time

            time.sleep(2.0)
    outs = [res.results[b]["out"] for b in range(B)]
    out = np.stack([np.asarray(o, dtype=np.float32) for o in outs], axis=0)
    return out, res


def kernel(dist_t, p_gen, alph_t, batch_vocab, pointer):
    out, _ = run(dist_t, p_gen, alph_t, batch_vocab, pointer)
    return out


# revision 17
# speedup vs baseline: 1.2426x; 1.1306x over previous
"""CopyNet extended-vocab projection kernel for Trainium2 (8 NeuronCores).

out[b, t, v] = p_gen[b,t] * pad(dist_t)[b,t,v] + (1 - p_gen[b,t]) * copyp[b,t,v]
copyp[b, t, v] = sum_{s: pointer[b,s]==v} alph_t[b, s, t]

Strategy: pure data-parallel over batch (B=8 -> 8 cores, one batch element per
core). All bulk I/O runs in bf16 (dist cast on host, output upcast on host,
alpha pre-scaled by (1 - p_gen) and cast on host). Per-core traffic ~33 MB;
the per-core HBM/fabric ceiling observed in traces is ~420 GB/s, so the
stream floor is ~80 us.

The copy term is a one-hot matmul on the tensor engine: onehot[s, v] =
(pointer[s] == v), built on-chip (iota + is_equal) once per 4096-wide vocab
macro-tile into a persistent SBUF arena during the prologue. Pointers are
host-grouped by owning macro-tile so each macro's matmul contracts over
K=128 grouped rows.

The PSUM drain + generation-term fuse is split across three engines so no
single engine paces the HBM stream (DVE at 1x mode needs ~2.3us per
2048-wide drain; 32 drains would exceed the stream budget):
  path_a: DVE scalar_tensor_tensor (out = dist*pgen + psum) directly.
  path_b: scalar-engine activation copies psum -> bf16 out slice, then
          GPSIMD does the in-place FMA (out = dist*pgen + out) in SBUF.
DMA dispatch cost (~0.6us per descriptor-generation instruction) is kept off
the critical path by using few, large DMAs: one combined alpha arena load,
one pgen load (host pre-layouts both), and [128, 4096] dist/out tiles.
The last macro's dist tiles are dedicated, tail-zeroed buffers so the pad
region beyond the true vocab needs no special-casing in the drain.

If any macro-tile owns more than 128 pointers (probability ~1e-9 for uniform
pointers), the kernel falls back to a dense K=512 f32 variant that makes no
assumption about pointer distribution.
"""
import sys

sys.path.insert(0, "/opt/trn_rl_repo")

import numpy as np

import concourse.bacc as bacc
import concourse.bass as bass
import concourse.tile as tile
from concourse import mybir
from concourse.bass_utils import run_bass_kernel_spmd

B = 8
L_DEC = 256
V = 32000
L_SRC = 512
V_EXT = 32128
P = 128
NCORES = 8
NPSUM = 512   # psum bank width at fp32

F32 = mybir.dt.float32
BF16 = mybir.dt.bfloat16
I16 = mybir.dt.int16
I32 = mybir.dt.int32

MACRO_SPARSE = 4096
HALF = 2048   # drain granularity (4 PSUM banks)
N_MACRO_SPARSE = (V_EXT + MACRO_SPARSE - 1) // MACRO_SPARSE  # 8 (last 3456)

_NC_CACHE = {}


def _build_nc_sparse():
    """bf16-I/O, K=128-per-macro-tile variant: host-grouped pointers."""
    nc = bacc.Bacc("TRN2", target_bir_lowering=False, debug=False)
    dist_d = nc.dram_tensor("dist", [L_DEC, V], BF16, kind="ExternalInput").ap()
    out_d = nc.dram_tensor("out", [L_DEC, V_EXT], BF16, kind="ExternalOutput").ap()
    # agh: q-scaled alpha rows pre-grouped by owning macro-tile on the host,
    # laid out [p, m*L_DEC + t] so one DMA loads the whole arena
    agh_d = nc.dram_tensor(
        "agh", [P, N_MACRO_SPARSE * L_DEC], BF16, kind="ExternalInput"
    ).ap()
    # smalls packs pgen (cols 0..1) and the grouped pointer values as f32
    # (cols 2..9), padded to 128 cols: 512 B per partition keeps the DMA
    # descriptors at line rate (8/16-byte descriptors take 7-12us to land
    # under early HBM contention and gated the whole drain pipeline)
    smalls_d = nc.dram_tensor(
        "smalls", [P, P], F32, kind="ExternalInput"
    ).ap()
    iota_d = nc.dram_tensor(
        "iota", [P, MACRO_SPARSE], I16, kind="ExternalInput"
    ).ap()

    n_tchunk = L_DEC // P
    MACRO = MACRO_SPARSE
    M_LAST = N_MACRO_SPARSE - 1
    DW_LAST = V - M_LAST * MACRO          # 3328 dist cols in last macro
    VW_LAST = V_EXT - M_LAST * MACRO      # 3456 out cols in last macro

    with tile.TileContext(nc) as tc:
        with (
            tc.tile_pool(name="const", bufs=1) as cpool,
            tc.tile_pool(name="dist", bufs=6) as dpool,
            tc.tile_pool(name="outp", bufs=4) as opool,
            tc.tile_pool(name="oh", bufs=2) as ohpool,
            tc.tile_pool(name="sh", bufs=2) as shpool,
            tc.tile_pool(name="psum", bufs=2, space="PSUM") as pspool,
        ):
            # --- prologue: one DMA per small input ---
            # iota leads the sync ring (HWDGE FIFO -> deterministic early
            # arrival ahead of the dist stream; it gates the first one-hot
            # and thereby the first store); ptr16 + pgen ride the scalar ring
            iota16 = cpool.tile([P, MACRO], I16)
            nc.sync.dma_start(iota16[:], iota_d[:])
            smalls_sb = cpool.tile([P, P], F32)
            nc.scalar.dma_start(smalls_sb[:], smalls_d[:])
            pgen_sb = smalls_sb[:, 0:n_tchunk]
            ptr_sb = smalls_sb[:, n_tchunk : n_tchunk + N_MACRO_SPARSE]
            agh_sb = cpool.tile([P, N_MACRO_SPARSE * L_DEC], BF16)
            nc.sync.dma_start(agh_sb[:], agh_d[:])

            # warm the ACT function table off the critical path (the first
            # ACTIVATE otherwise pays a ~1.3us lazy table load mid-pipeline)
            warm = shpool.tile([P, 1], F32, tag="warm")
            nc.scalar.activation(
                out=warm[:], in_=smalls_sb[:, 0:1],
                func=mybir.ActivationFunctionType.Copy, scale=1.0,
            )

            # last macro's dist tiles: dedicated buffers with the pad region
            # beyond the true vocab zeroed once, so every drain is a plain FMA
            dist_last = []
            for t in range(n_tchunk):
                dl = cpool.tile([P, MACRO], BF16, tag=f"dlast{t}")
                nc.gpsimd.memset(dl[:, DW_LAST:], 0.0)
                dist_last.append(dl)

            # diag(pgen) per t-chunk for the tensor-engine generation-term
            # accumulate (identity from iota with channel_multiplier=-1,
            # scaled per-partition by pgen)
            iota_id = cpool.tile([P, P], I16)
            nc.gpsimd.iota(iota_id[:], pattern=[[1, P]], base=0,
                           channel_multiplier=-1)
            diag_pg = []
            for t in range(n_tchunk):
                dg = cpool.tile([P, P], BF16, tag=f"diag{t}")
                nc.vector.tensor_scalar(
                    out=dg[:], in0=iota_id[:],
                    scalar1=0.0, scalar2=smalls_sb[:, t : t + 1],
                    op0=mybir.AluOpType.is_equal,
                    op1=mybir.AluOpType.mult,
                )
                diag_pg.append(dg)

            # --- main loop: one [128, 4096] tile per (macro, t-chunk) ---
            # drain pipeline per 2048-wide half: tensor engine -> ACT copies
            # PSUM into the out tile (bf16) -> DVE adds the pre-scaled dist
            # in place (tensor_tensor runs in 2x mode on all-bf16 operands).
            # dist is pre-scaled by pgen once per tile on the DVE (4x mode),
            # so the per-element FMA never runs at the DVE's 1x PSUM rate.
            for m in range(N_MACRO_SPARSE):
                v0 = m * MACRO
                vw = min(MACRO, V_EXT - v0)
                dw = max(0, min(vw, V - v0))
                # build this macro's one-hot lazily (not all upfront): the
                # Tile scheduler expresses store waits as counting-semaphore
                # thresholds over the Vector queue, so any straggling oh
                # build queued ahead of the TTs would gate EVERY store
                shift = shpool.tile([P, 1], F32, tag="shift")
                nc.vector.tensor_scalar(
                    out=shift[:],
                    in0=smalls_sb[:, n_tchunk + m : n_tchunk + m + 1],
                    scalar1=float(m * MACRO),
                    scalar2=None, op0=mybir.AluOpType.subtract,
                )
                oh = ohpool.tile([P, MACRO], BF16, tag="oh")
                nc.vector.tensor_scalar(
                    out=oh[:, :vw], in0=iota16[:, :vw],
                    scalar1=shift[:, 0:1], scalar2=None,
                    op0=mybir.AluOpType.is_equal,
                )
                for t in range(n_tchunk):
                    trow = slice(t * P, (t + 1) * P)
                    if m == M_LAST:
                        dist_sb = dist_last[t]
                        nc.sync.dma_start(
                            dist_sb[:, :dw], dist_d[trow, v0 : v0 + dw]
                        )
                    else:
                        dist_sb = dpool.tile([P, MACRO], BF16, tag="dist")
                        nc.sync.dma_start(
                            dist_sb[:], dist_d[trow, v0 : v0 + MACRO]
                        )
                    out_sb = opool.tile([P, MACRO], BF16, tag="out")
                    for h in range(2):
                        c0 = h * HALF
                        hw = min(HALF, vw - c0)   # 2048, or 1408 for m=7 h=1
                        if hw <= 0:
                            continue
                        # alternate drains between two INDEPENDENT paths so
                        # neither engine paces the stream and no half needs
                        # a cross-engine chain:
                        #   path_P: tensor engine also accumulates
                        #     diag(pgen)@dist into PSUM; drain is a pure ACT
                        #     copy (no DVE work at all)
                        #   path_a: DVE drains directly with a fused FMA
                        #     (out = dist*pgen + psum) at its 1x PSUM rate
                        idx = (m * n_tchunk + t) * 2 + h
                        path_P = idx % 2 == 0
                        psum = pspool.tile([P, HALF], F32, space="PSUM")
                        nj = (hw + NPSUM - 1) // NPSUM
                        for j in range(nj):
                            jw = min(NPSUM, hw - j * NPSUM)
                            nc.tensor.matmul(
                                out=psum[:, j * NPSUM : j * NPSUM + jw],
                                lhsT=agh_sb[:, m * L_DEC + t * P : m * L_DEC + (t + 1) * P],
                                rhs=oh[:, c0 + j * NPSUM : c0 + j * NPSUM + jw],
                                start=True, stop=not path_P,
                            )
                            if path_P:
                                nc.tensor.matmul(
                                    out=psum[:, j * NPSUM : j * NPSUM + jw],
                                    lhsT=diag_pg[t][:],
                                    rhs=dist_sb[:, c0 + j * NPSUM : c0 + j * NPSUM + jw],
                                    start=False, stop=True,
                                )
                        if path_P:
                            nc.scalar.activation(
                                out=out_sb[:, c0 : c0 + hw],
                                in_=psum[:, :hw],
                                func=mybir.ActivationFunctionType.Copy,
                                scale=1.0,
                            )
                        else:
                            nc.vector.scalar_tensor_tensor(
                                out=out_sb[:, c0 : c0 + hw],
                                in0=dist_sb[:, c0 : c0 + hw],
                                scalar=smalls_sb[:, t : t + 1],
                                in1=psum[:, :hw],
                                op0=mybir.AluOpType.mult,
                                op1=mybir.AluOpType.add,
                            )
                    # stores ride the GPSIMD SWDGE queue: loads own the sync
                    # HWDGE ring and ACT drains own the scalar queue, so no
                    # store dispatch can convoy-block either (HWDGE rings are
                    # FIFO per issuing engine)
                    nc.gpsimd.dma_start(
                        out_d[trow, v0 : v0 + vw], out_sb[:, :vw]
                    )
    nc.compile()
    return nc


def _build_nc_dense():
    """Dense K=512 f32 fallback: no assumption on pointer distribution."""
    MACRO = 2048
    nc = bacc.Bacc("TRN2", target_bir_lowering=False, debug=False)
    dist_d = nc.dram_tensor("dist", [L_DEC, V], F32, kind="ExternalInput").ap()
    pgen_d = nc.dram_tensor("pgen", [L_DEC, 1], F32, kind="ExternalInput").ap()
    alpha_d = nc.dram_tensor("alpha", [L_SRC, L_DEC], F32, kind="ExternalInput").ap()
    out_d = nc.dram_tensor("out", [L_DEC, V_EXT], F32, kind="ExternalOutput").ap()
    ptr_d = nc.dram_tensor("ptr", [L_SRC, 1], I32, kind="ExternalInput").ap()

    n_schunk = L_SRC // P
    n_tchunk = L_DEC // P
    n_macro = (V_EXT + MACRO - 1) // MACRO

    with tile.TileContext(nc) as tc:
        with (
            tc.tile_pool(name="const", bufs=1) as cpool,
            tc.tile_pool(name="dist", bufs=3) as dpool,
            tc.tile_pool(name="outp", bufs=3) as opool,
            tc.tile_pool(name="oh", bufs=2) as ohpool,
            tc.tile_pool(name="psum", bufs=6, space="PSUM") as pspool,
        ):
            ptr_sb = cpool.tile([P, n_schunk], I32)
            for c in range(n_schunk):
                nc.sync.dma_start(ptr_sb[:, c : c + 1], ptr_d[c * P : (c + 1) * P, 0:1])
            pgen_sb = cpool.tile([P, n_tchunk], F32)
            for t in range(n_tchunk):
                nc.sync.dma_start(
                    pgen_sb[:, t : t + 1], pgen_d[t * P : (t + 1) * P, 0:1]
                )
            q_sb = cpool.tile([P, n_tchunk], F32)
            nc.vector.tensor_scalar(
                out=q_sb[:], in0=pgen_sb[:], scalar1=-1.0, scalar2=1.0,
                op0=mybir.AluOpType.mult, op1=mybir.AluOpType.add,
            )
            alpha_terms = []  # per chunk: (hi, mid, lo) bf16
            for c in range(n_schunk):
                a = cpool.tile([P, L_DEC], F32, tag=f"alpha{c}")
                nc.sync.dma_start(a[:], alpha_d[c * P : (c + 1) * P, :])
                hi = cpool.tile([P, L_DEC], BF16, tag=f"ahi{c}")
                nc.vector.tensor_copy(hi[:], a[:])
                r1 = cpool.tile([P, L_DEC], F32, tag=f"r1{c}")
                nc.vector.tensor_tensor(
                    out=r1[:], in0=a[:], in1=hi[:], op=mybir.AluOpType.subtract
                )
                mid = cpool.tile([P, L_DEC], BF16, tag=f"amid{c}")
                nc.vector.tensor_copy(mid[:], r1[:])
                lo = cpool.tile([P, L_DEC], BF16, tag=f"alo{c}")
                nc.vector.tensor_tensor(
                    out=lo[:], in0=r1[:], in1=mid[:], op=mybir.AluOpType.subtract
                )
                alpha_terms.append((hi, mid, lo))
            iota16 = cpool.tile([P, MACRO], I16)
            nc.gpsimd.iota(iota16[:], pattern=[[1, MACRO]], base=0, channel_multiplier=0)

            for m in range(n_macro):
                v0 = m * MACRO
                vw = min(MACRO, V_EXT - v0)
                dw = max(0, min(vw, V - v0))
                shift = ohpool.tile([P, n_schunk], F32, tag="shift")
                nc.vector.tensor_scalar(
                    out=shift[:], in0=ptr_sb[:], scalar1=float(v0), scalar2=None,
                    op0=mybir.AluOpType.subtract,
                )
                ohs = []
                for c in range(n_schunk):
                    oh = ohpool.tile([P, MACRO], BF16, tag=f"oh{c}")
                    nc.vector.tensor_scalar(
                        out=oh[:, :vw], in0=iota16[:, :vw],
                        scalar1=shift[:, c : c + 1], scalar2=None,
                        op0=mybir.AluOpType.is_equal,
                    )
                    ohs.append(oh)
                for t in range(n_tchunk):
                    trow = slice(t * P, (t + 1) * P)
                    dist_sb = dpool.tile([P, MACRO], F32, tag="dist")
                    if dw > 0:
                        nc.sync.dma_start(dist_sb[:, :dw], dist_d[trow, v0 : v0 + dw])
                    out_sb = opool.tile([P, MACRO], F32, tag="out")
                    nj = (vw + NPSUM - 1) // NPSUM
                    for j in range(nj):
                        jw = min(NPSUM, vw - j * NPSUM)
                        psum = pspool.tile([P, NPSUM], F32, space="PSUM")
                        mm_list = [
                            (c, amat)
                            for term in range(3)
                            for c in range(n_schunk)
                            for amat in (alpha_terms[c][term],)
                        ]
                        for k, (c, amat) in enumerate(mm_list):
                            nc.tensor.matmul(
                                out=psum[:, :jw],
                                lhsT=amat[:, trow],
                                rhs=ohs[c][:, j * NPSUM : j * NPSUM + jw],
                                start=(k == 0), stop=(k == len(mm_list) - 1),
                            )
                        nc.scalar.activation(
                            out=out_sb[:, j * NPSUM : j * NPSUM + jw],
                            in_=psum[:, :jw],
                            func=mybir.ActivationFunctionType.Copy,
                            scale=q_sb[:, t : t + 1],
                        )
                    if dw > 0:
                        nc.vector.scalar_tensor_tensor(
                            out=out_sb[:, :dw], in0=dist_sb[:, :dw],
                            scalar=pgen_sb[:, t : t + 1], in1=out_sb[:, :dw],
                            op0=mybir.AluOpType.mult, op1=mybir.AluOpType.add,
                        )
                    nc.sync.dma_start(out_d[trow, v0 : v0 + vw], out_sb[:, :vw])
    nc.compile()
    return nc


def _get_nc(variant):
    if variant not in _NC_CACHE:
        _NC_CACHE[variant] = (
            _build_nc_sparse() if variant == "sparse" else _build_nc_dense()
        )
    return _NC_CACHE[variant]


_IOTA = None


def _iota_const():
    global _IOTA
    if _IOTA is None:
        _IOTA = np.ascontiguousarray(
            np.broadcast_to(
                np.arange(MACRO_SPARSE, dtype=np.int16), (P, MACRO_SPARSE)
            )
        )
    return _IOTA


def _bf16():
    import ml_dtypes

    return ml_dtypes.bfloat16


def _group_pointers(ptr_b):
    """Group source indices by owning macro-tile. Returns (idx, ptrg) each
    [N_MACRO_SPARSE, P, 1] int32, or None if any tile owns > P pointers."""
    owner = ptr_b // MACRO_SPARSE
    idx = np.zeros((N_MACRO_SPARSE, P, 1), np.int32)
    ptrg = np.full((N_MACRO_SPARSE, P, 1), -1, np.int32)
    for m in range(N_MACRO_SPARSE):
        sel = np.nonzero(owner == m)[0]
        if len(sel) > P:
            return None, None
        idx[m, : len(sel), 0] = sel
        ptrg[m, : len(sel), 0] = ptr_b[sel]
    return idx, ptrg


def _prep(dist_t, p_gen, alph_t, pointer):
    dist_t = np.asarray(dist_t, dtype=np.float32)
    p_gen = np.ascontiguousarray(
        np.asarray(p_gen, dtype=np.float32).reshape(B, L_DEC, 1)
    )
    alph_t = np.asarray(alph_t, dtype=np.float32)
    ptr = np.asarray(pointer).astype(np.int32).reshape(B, L_SRC)
    assert dist_t.shape == (B, L_DEC, V), dist_t.shape
    assert alph_t.shape == (B, L_SRC, L_DEC), alph_t.shape

    in_maps = []
    variant = "sparse"
    metas = []
    for b in range(B):
        idx, ptrg = _group_pointers(ptr[b])
        if idx is None:
            variant = "dense"
            break
        metas.append((idx, ptrg))
    if variant == "sparse":
        bf16 = _bf16()
        dist_bf = np.ascontiguousarray(dist_t.astype(bf16))
        # fold (1 - p_gen) into alpha before the bf16 round
        alphaq = (alph_t * (1.0 - p_gen.transpose(0, 2, 1))).astype(bf16)
        n_tchunk = L_DEC // P
        in_maps = []
        for b in range(B):
            idx, ptrg = metas[b]
            # gather alpha rows by owning macro on the host; zero the
            # padding rows so they contribute nothing to the matmul
            alphag = alphaq[b][idx[:, :, 0]]          # [N_MACRO, P, L_DEC]
            alphag[ptrg[:, :, 0] < 0] = 0
            # [p, m*L_DEC + t] layout -> one DMA loads the whole arena
            agh = np.ascontiguousarray(
                alphag.transpose(1, 0, 2).reshape(P, N_MACRO_SPARSE * L_DEC)
            )
            # smalls[p, 0:2] = pgen per t-chunk, smalls[p, 2:10] = grouped
            # pointer values as f32, padded to 128 cols for line-rate DMA
            smalls = np.zeros((P, P), np.float32)
            smalls[:, :n_tchunk] = p_gen[b, :, 0].reshape(n_tchunk, P).T
            smalls[:, n_tchunk : n_tchunk + N_MACRO_SPARSE] = (
                ptrg[:, :, 0].T.astype(np.float32)
            )
            in_maps.append(
                {"dist": dist_bf[b], "smalls": smalls, "agh": agh,
                 "iota": _iota_const()}
            )
    else:
        dist_f32 = np.ascontiguousarray(dist_t)
        alph_f32 = np.ascontiguousarray(alph_t)
        in_maps = [
            {"dist": dist_f32[b], "pgen": p_gen[b], "alpha": alph_f32[b],
             "ptr": np.ascontiguousarray(ptr[b].reshape(L_SRC, 1))}
            for b in range(B)
        ]
    return variant, in_maps


def run(dist_t, p_gen, alph_t, batch_vocab, pointer, trace=False,
        force_variant=None, **spmd_kwargs):
    """Run the kernel; returns (output, BassKernelResults)."""
    assert batch_vocab.shape[0] == V_EXT
    variant, in_maps = _prep(dist_t, p_gen, alph_t, pointer)
    if force_variant == "dense" and variant == "sparse":
        ptr = np.asarray(pointer).astype(np.int32).reshape(B, L_SRC)
        dist_f32 = np.ascontiguousarray(np.asarray(dist_t, dtype=np.float32))
        alph_f32 = np.ascontiguousarray(np.asarray(alph_t, dtype=np.float32))
        p_gen_f = np.ascontiguousarray(
            np.asarray(p_gen, dtype=np.float32).reshape(B, L_DEC, 1)
        )
        in_maps = [
            {"dist": dist_f32[b], "pgen": p_gen_f[b], "alpha": alph_f32[b],
             "ptr": np.ascontiguousarray(ptr[b].reshape(L_SRC, 1))}
            for b in range(B)
        ]
        variant = "dense"
    run.last_variant = variant
    res = None
    for attempt in range(3):
        try:
            res = run_bass_kernel_spmd(
                _get_nc(variant), in_maps, list(range(NCORES)),
                trace=trace and attempt == 0, **spmd_kwargs
            )
            break
        except Exception:
            # transient device-state failures (e.g. NRT_EXEC_UNIT_UNRECOVERABLE
            # left over from a previous profiled session) sometimes clear on
            # retry; give it two more chances (untraced -- profiling itself
            # can be the destabilizer) before giving up
            if attempt == 2:
                raise
            import time

            time.sleep(2.0)
    outs = [res.results[b]["out"] for b in range(B)]
    out = np.stack([np.asarray(o, dtype=np.float32) for o in outs], axis=0)
    return out, res


def kernel(dist_t, p_gen, alph_t, batch_vocab, pointer):
    out, _ = run(dist_t, p_gen, alph_t, batch_vocab, pointer)
    return out


# revision 18
# speedup vs baseline: 1.2606x; 1.0144x over previous
"""CopyNet extended-vocab projection kernel for Trainium2 (8 NeuronCores).

out[b, t, v] = p_gen[b,t] * pad(dist_t)[b,t,v] + (1 - p_gen[b,t]) * copyp[b,t,v]
copyp[b, t, v] = sum_{s: pointer[b,s]==v} alph_t[b, s, t]

Strategy: pure data-parallel over batch (B=8 -> 8 cores, one batch element per
core). All bulk I/O runs in bf16 (dist cast on host, output upcast on host,
alpha pre-scaled by (1 - p_gen) and cast on host). Per-core traffic ~33 MB;
the per-core HBM/fabric ceiling observed in traces is ~420 GB/s, so the
stream floor is ~80 us.

The copy term is a one-hot matmul on the tensor engine: onehot[s, v] =
(pointer[s] == v), built on-chip (iota + is_equal) once per 4096-wide vocab
macro-tile into a persistent SBUF arena during the prologue. Pointers are
host-grouped by owning macro-tile so each macro's matmul contracts over
K=128 grouped rows.

The PSUM drain + generation-term fuse is split across three engines so no
single engine paces the HBM stream (DVE at 1x mode needs ~2.3us per
2048-wide drain; 32 drains would exceed the stream budget):
  path_a: DVE scalar_tensor_tensor (out = dist*pgen + psum) directly.
  path_b: scalar-engine activation copies psum -> bf16 out slice, then
          GPSIMD does the in-place FMA (out = dist*pgen + out) in SBUF.
DMA dispatch cost (~0.6us per descriptor-generation instruction) is kept off
the critical path by using few, large DMAs: one combined alpha arena load,
one pgen load (host pre-layouts both), and [128, 4096] dist/out tiles.
The last macro's dist tiles are dedicated, tail-zeroed buffers so the pad
region beyond the true vocab needs no special-casing in the drain.

If any macro-tile owns more than 128 pointers (probability ~1e-9 for uniform
pointers), the kernel falls back to a dense K=512 f32 variant that makes no
assumption about pointer distribution.
"""
import sys

sys.path.insert(0, "/opt/trn_rl_repo")

import numpy as np

import concourse.bacc as bacc
import concourse.bass as bass
import concourse.tile as tile
from concourse import mybir
from concourse.bass_utils import run_bass_kernel_spmd

B = 8
L_DEC = 256
V = 32000
L_SRC = 512
V_EXT = 32128
P = 128
NCORES = 8
NPSUM = 512   # psum bank width at fp32

F32 = mybir.dt.float32
BF16 = mybir.dt.bfloat16
I16 = mybir.dt.int16
I32 = mybir.dt.int32

MACRO_SPARSE = 4096
HALF = 2048   # drain granularity (4 PSUM banks)
N_MACRO_SPARSE = (V_EXT + MACRO_SPARSE - 1) // MACRO_SPARSE  # 8 (last 3456)

_NC_CACHE = {}


def _build_nc_sparse():
    """bf16-I/O, K=128-per-macro-tile variant: host-grouped pointers."""
    nc = bacc.Bacc("TRN2", target_bir_lowering=False, debug=False)
    dist_d = nc.dram_tensor("dist", [L_DEC, V], BF16, kind="ExternalInput").ap()
    out_d = nc.dram_tensor("out", [L_DEC, V_EXT], BF16, kind="ExternalOutput").ap()
    # agh: q-scaled alpha rows pre-grouped by owning macro-tile on the host,
    # laid out [p, m*L_DEC + t] so one DMA loads the whole arena
    agh_d = nc.dram_tensor(
        "agh", [P, N_MACRO_SPARSE * L_DEC], BF16, kind="ExternalInput"
    ).ap()
    # smalls packs pgen (cols 0..1) and the grouped pointer values as f32
    # (cols 2..9), padded to 128 cols: 512 B per partition keeps the DMA
    # descriptors at line rate (8/16-byte descriptors take 7-12us to land
    # under early HBM contention and gated the whole drain pipeline)
    smalls_d = nc.dram_tensor(
        "smalls", [P, P], F32, kind="ExternalInput"
    ).ap()

    n_tchunk = L_DEC // P
    MACRO = MACRO_SPARSE
    M_LAST = N_MACRO_SPARSE - 1
    DW_LAST = V - M_LAST * MACRO          # 3328 dist cols in last macro
    VW_LAST = V_EXT - M_LAST * MACRO      # 3456 out cols in last macro

    with tile.TileContext(nc) as tc:
        with (
            tc.tile_pool(name="const", bufs=1) as cpool,
            tc.tile_pool(name="dist", bufs=6) as dpool,
            tc.tile_pool(name="outp", bufs=4) as opool,
            tc.tile_pool(name="oh", bufs=2) as ohpool,
            tc.tile_pool(name="sh", bufs=2) as shpool,
            tc.tile_pool(name="psum", bufs=2, space="PSUM") as pspool,
        ):
            # --- prologue ---
            # iota is generated on-chip (GPSIMD, ~7us, done before the first
            # store dispatch needs the queue) -- costs zero HBM bytes; the
            # packed smalls tensor rides the scalar ring
            iota16 = cpool.tile([P, MACRO], I16)
            nc.gpsimd.iota(iota16[:], pattern=[[1, MACRO]], base=0,
                           channel_multiplier=0)
            smalls_sb = cpool.tile([P, P], F32)
            nc.scalar.dma_start(smalls_sb[:], smalls_d[:])
            pgen_sb = smalls_sb[:, 0:n_tchunk]
            ptr_sb = smalls_sb[:, n_tchunk : n_tchunk + N_MACRO_SPARSE]
            agh_sb = cpool.tile([P, N_MACRO_SPARSE * L_DEC], BF16)
            nc.sync.dma_start(agh_sb[:], agh_d[:])

            # warm the ACT function table off the critical path (the first
            # ACTIVATE otherwise pays a ~1.3us lazy table load mid-pipeline)
            warm = shpool.tile([P, 1], F32, tag="warm")
            nc.scalar.activation(
                out=warm[:], in_=smalls_sb[:, 0:1],
                func=mybir.ActivationFunctionType.Copy, scale=1.0,
            )

            # last macro's dist tiles: dedicated buffers with the pad region
            # beyond the true vocab zeroed once, so every drain is a plain FMA
            dist_last = []
            for t in range(n_tchunk):
                dl = cpool.tile([P, MACRO], BF16, tag=f"dlast{t}")
                nc.gpsimd.memset(dl[:, DW_LAST:], 0.0)
                dist_last.append(dl)

            # diag(pgen) per t-chunk for the tensor-engine generation-term
            # accumulate (identity from iota with channel_multiplier=-1,
            # scaled per-partition by pgen)
            iota_id = cpool.tile([P, P], I16)
            nc.gpsimd.iota(iota_id[:], pattern=[[1, P]], base=0,
                           channel_multiplier=-1)
            diag_pg = []
            for t in range(n_tchunk):
                dg = cpool.tile([P, P], BF16, tag=f"diag{t}")
                nc.vector.tensor_scalar(
                    out=dg[:], in0=iota_id[:],
                    scalar1=0.0, scalar2=smalls_sb[:, t : t + 1],
                    op0=mybir.AluOpType.is_equal,
                    op1=mybir.AluOpType.mult,
                )
                diag_pg.append(dg)

            # --- main loop: one [128, 4096] tile per (macro, t-chunk) ---
            # drain pipeline per 2048-wide half: tensor engine -> ACT copies
            # PSUM into the out tile (bf16) -> DVE adds the pre-scaled dist
            # in place (tensor_tensor runs in 2x mode on all-bf16 operands).
            # dist is pre-scaled by pgen once per tile on the DVE (4x mode),
            # so the per-element FMA never runs at the DVE's 1x PSUM rate.
            for m in range(N_MACRO_SPARSE):
                v0 = m * MACRO
                vw = min(MACRO, V_EXT - v0)
                dw = max(0, min(vw, V - v0))
                # build this macro's one-hot lazily (not all upfront): the
                # Tile scheduler expresses store waits as counting-semaphore
                # thresholds over the Vector queue, so any straggling oh
                # build queued ahead of the TTs would gate EVERY store
                shift = shpool.tile([P, 1], F32, tag="shift")
                nc.vector.tensor_scalar(
                    out=shift[:],
                    in0=smalls_sb[:, n_tchunk + m : n_tchunk + m + 1],
                    scalar1=float(m * MACRO),
                    scalar2=None, op0=mybir.AluOpType.subtract,
                )
                oh = ohpool.tile([P, MACRO], BF16, tag="oh")
                nc.vector.tensor_scalar(
                    out=oh[:, :vw], in0=iota16[:, :vw],
                    scalar1=shift[:, 0:1], scalar2=None,
                    op0=mybir.AluOpType.is_equal,
                )
                for t in range(n_tchunk):
                    trow = slice(t * P, (t + 1) * P)
                    if m == M_LAST:
                        dist_sb = dist_last[t]
                        nc.sync.dma_start(
                            dist_sb[:, :dw], dist_d[trow, v0 : v0 + dw]
                        )
                    else:
                        dist_sb = dpool.tile([P, MACRO], BF16, tag="dist")
                        nc.sync.dma_start(
                            dist_sb[:], dist_d[trow, v0 : v0 + MACRO]
                        )
                    out_sb = opool.tile([P, MACRO], BF16, tag="out")
                    for h in range(2):
                        c0 = h * HALF
                        hw = min(HALF, vw - c0)   # 2048, or 1408 for m=7 h=1
                        if hw <= 0:
                            continue
                        # alternate drains between two INDEPENDENT paths so
                        # neither engine paces the stream and no half needs
                        # a cross-engine chain:
                        #   path_P: tensor engine also accumulates
                        #     diag(pgen)@dist into PSUM; drain is a pure ACT
                        #     copy (no DVE work at all)
                        #   path_a: DVE drains directly with a fused FMA
                        #     (out = dist*pgen + psum) at its 1x PSUM rate
                        idx = (m * n_tchunk + t) * 2 + h
                        path_P = idx % 8 in (0, 3, 6)   # 12 of 32 halves
                        psum = pspool.tile([P, HALF], F32, space="PSUM")
                        nj = (hw + NPSUM - 1) // NPSUM
                        # two passes (all oh-matmuls, then all diag-matmuls)
                        # so the stationary lhsT switches once per half, not
                        # once per 512-slice
                        for j in range(nj):
                            jw = min(NPSUM, hw - j * NPSUM)
                            nc.tensor.matmul(
                                out=psum[:, j * NPSUM : j * NPSUM + jw],
                                lhsT=agh_sb[:, m * L_DEC + t * P : m * L_DEC + (t + 1) * P],
                                rhs=oh[:, c0 + j * NPSUM : c0 + j * NPSUM + jw],
                                start=True, stop=not path_P,
                            )
                        if path_P:
                            for j in range(nj):
                                jw = min(NPSUM, hw - j * NPSUM)
                                nc.tensor.matmul(
                                    out=psum[:, j * NPSUM : j * NPSUM + jw],
                                    lhsT=diag_pg[t][:],
                                    rhs=dist_sb[:, c0 + j * NPSUM : c0 + j * NPSUM + jw],
                                    start=False, stop=True,
                                )
                        if path_P:
                            nc.scalar.activation(
                                out=out_sb[:, c0 : c0 + hw],
                                in_=psum[:, :hw],
                                func=mybir.ActivationFunctionType.Copy,
                                scale=1.0,
                            )
                        else:
                            nc.vector.scalar_tensor_tensor(
                                out=out_sb[:, c0 : c0 + hw],
                                in0=dist_sb[:, c0 : c0 + hw],
                                scalar=smalls_sb[:, t : t + 1],
                                in1=psum[:, :hw],
                                op0=mybir.AluOpType.mult,
                                op1=mybir.AluOpType.add,
                            )
                    # stores ride the GPSIMD SWDGE queue: loads own the sync
                    # HWDGE ring and ACT drains own the scalar queue, so no
                    # store dispatch can convoy-block either (HWDGE rings are
                    # FIFO per issuing engine)
                    nc.gpsimd.dma_start(
                        out_d[trow, v0 : v0 + vw], out_sb[:, :vw]
                    )
    nc.compile()
    return nc


def _build_nc_dense():
    """Dense K=512 f32 fallback: no assumption on pointer distribution."""
    MACRO = 2048
    nc = bacc.Bacc("TRN2", target_bir_lowering=False, debug=False)
    dist_d = nc.dram_tensor("dist", [L_DEC, V], F32, kind="ExternalInput").ap()
    pgen_d = nc.dram_tensor("pgen", [L_DEC, 1], F32, kind="ExternalInput").ap()
    alpha_d = nc.dram_tensor("alpha", [L_SRC, L_DEC], F32, kind="ExternalInput").ap()
    out_d = nc.dram_tensor("out", [L_DEC, V_EXT], F32, kind="ExternalOutput").ap()
    ptr_d = nc.dram_tensor("ptr", [L_SRC, 1], I32, kind="ExternalInput").ap()

    n_schunk = L_SRC // P
    n_tchunk = L_DEC // P
    n_macro = (V_EXT + MACRO - 1) // MACRO

    with tile.TileContext(nc) as tc:
        with (
            tc.tile_pool(name="const", bufs=1) as cpool,
            tc.tile_pool(name="dist", bufs=3) as dpool,
            tc.tile_pool(name="outp", bufs=3) as opool,
            tc.tile_pool(name="oh", bufs=2) as ohpool,
            tc.tile_pool(name="psum", bufs=6, space="PSUM") as pspool,
        ):
            ptr_sb = cpool.tile([P, n_schunk], I32)
            for c in range(n_schunk):
                nc.sync.dma_start(ptr_sb[:, c : c + 1], ptr_d[c * P : (c + 1) * P, 0:1])
            pgen_sb = cpool.tile([P, n_tchunk], F32)
            for t in range(n_tchunk):
                nc.sync.dma_start(
                    pgen_sb[:, t : t + 1], pgen_d[t * P : (t + 1) * P, 0:1]
                )
            q_sb = cpool.tile([P, n_tchunk], F32)
            nc.vector.tensor_scalar(
                out=q_sb[:], in0=pgen_sb[:], scalar1=-1.0, scalar2=1.0,
                op0=mybir.AluOpType.mult, op1=mybir.AluOpType.add,
            )
            alpha_terms = []  # per chunk: (hi, mid, lo) bf16
            for c in range(n_schunk):
                a = cpool.tile([P, L_DEC], F32, tag=f"alpha{c}")
                nc.sync.dma_start(a[:], alpha_d[c * P : (c + 1) * P, :])
                hi = cpool.tile([P, L_DEC], BF16, tag=f"ahi{c}")
                nc.vector.tensor_copy(hi[:], a[:])
                r1 = cpool.tile([P, L_DEC], F32, tag=f"r1{c}")
                nc.vector.tensor_tensor(
                    out=r1[:], in0=a[:], in1=hi[:], op=mybir.AluOpType.subtract
                )
                mid = cpool.tile([P, L_DEC], BF16, tag=f"amid{c}")
                nc.vector.tensor_copy(mid[:], r1[:])
                lo = cpool.tile([P, L_DEC], BF16, tag=f"alo{c}")
                nc.vector.tensor_tensor(
                    out=lo[:], in0=r1[:], in1=mid[:], op=mybir.AluOpType.subtract
                )
                alpha_terms.append((hi, mid, lo))
            iota16 = cpool.tile([P, MACRO], I16)
            nc.gpsimd.iota(iota16[:], pattern=[[1, MACRO]], base=0, channel_multiplier=0)

            for m in range(n_macro):
                v0 = m * MACRO
                vw = min(MACRO, V_EXT - v0)
                dw = max(0, min(vw, V - v0))
                shift = ohpool.tile([P, n_schunk], F32, tag="shift")
                nc.vector.tensor_scalar(
                    out=shift[:], in0=ptr_sb[:], scalar1=float(v0), scalar2=None,
                    op0=mybir.AluOpType.subtract,
                )
                ohs = []
                for c in range(n_schunk):
                    oh = ohpool.tile([P, MACRO], BF16, tag=f"oh{c}")
                    nc.vector.tensor_scalar(
                        out=oh[:, :vw], in0=iota16[:, :vw],
                        scalar1=shift[:, c : c + 1], scalar2=None,
                        op0=mybir.AluOpType.is_equal,
                    )
                    ohs.append(oh)
                for t in range(n_tchunk):
                    trow = slice(t * P, (t + 1) * P)
                    dist_sb = dpool.tile([P, MACRO], F32, tag="dist")
                    if dw > 0:
                        nc.sync.dma_start(dist_sb[:, :dw], dist_d[trow, v0 : v0 + dw])
                    out_sb = opool.tile([P, MACRO], F32, tag="out")
                    nj = (vw + NPSUM - 1) // NPSUM
                    for j in range(nj):
                        jw = min(NPSUM, vw - j * NPSUM)
                        psum = pspool.tile([P, NPSUM], F32, space="PSUM")
                        mm_list = [
                            (c, amat)
                            for term in range(3)
                            for c in range(n_schunk)
                            for amat in (alpha_terms[c][term],)
                        ]
                        for k, (c, amat) in enumerate(mm_list):
                            nc.tensor.matmul(
                                out=psum[:, :jw],
                                lhsT=amat[:, trow],
                                rhs=ohs[c][:, j * NPSUM : j * NPSUM + jw],
                                start=(k == 0), stop=(k == len(mm_list) - 1),
                            )
                        nc.scalar.activation(
                            out=out_sb[:, j * NPSUM : j * NPSUM + jw],
                            in_=psum[:, :jw],
                            func=mybir.ActivationFunctionType.Copy,
                            scale=q_sb[:, t : t + 1],
                        )
                    if dw > 0:
                        nc.vector.scalar_tensor_tensor(
                            out=out_sb[:, :dw], in0=dist_sb[:, :dw],
                            scalar=pgen_sb[:, t : t + 1], in1=out_sb[:, :dw],
                            op0=mybir.AluOpType.mult, op1=mybir.AluOpType.add,
                        )
                    nc.sync.dma_start(out_d[trow, v0 : v0 + vw], out_sb[:, :vw])
    nc.compile()
    return nc


def _get_nc(variant):
    if variant not in _NC_CACHE:
        _NC_CACHE[variant] = (
            _build_nc_sparse() if variant == "sparse" else _build_nc_dense()
        )
    return _NC_CACHE[variant]


_IOTA = None


def _iota_const():
    global _IOTA
    if _IOTA is None:
        _IOTA = np.ascontiguousarray(
            np.broadcast_to(
                np.arange(MACRO_SPARSE, dtype=np.int16), (P, MACRO_SPARSE)
            )
        )
    return _IOTA


def _bf16():
    import ml_dtypes

    return ml_dtypes.bfloat16


def _group_pointers(ptr_b):
    """Group source indices by owning macro-tile. Returns (idx, ptrg) each
    [N_MACRO_SPARSE, P, 1] int32, or None if any tile owns > P pointers."""
    owner = ptr_b // MACRO_SPARSE
    idx = np.zeros((N_MACRO_SPARSE, P, 1), np.int32)
    ptrg = np.full((N_MACRO_SPARSE, P, 1), -1, np.int32)
    for m in range(N_MACRO_SPARSE):
        sel = np.nonzero(owner == m)[0]
        if len(sel) > P:
            return None, None
        idx[m, : len(sel), 0] = sel
        ptrg[m, : len(sel), 0] = ptr_b[sel]
    return idx, ptrg


def _prep(dist_t, p_gen, alph_t, pointer):
    dist_t = np.asarray(dist_t, dtype=np.float32)
    p_gen = np.ascontiguousarray(
        np.asarray(p_gen, dtype=np.float32).reshape(B, L_DEC, 1)
    )
    alph_t = np.asarray(alph_t, dtype=np.float32)
    ptr = np.asarray(pointer).astype(np.int32).reshape(B, L_SRC)
    assert dist_t.shape == (B, L_DEC, V), dist_t.shape
    assert alph_t.shape == (B, L_SRC, L_DEC), alph_t.shape

    in_maps = []
    variant = "sparse"
    metas = []
    for b in range(B):
        idx, ptrg = _group_pointers(ptr[b])
        if idx is None:
            variant = "dense"
            break
        metas.append((idx, ptrg))
    if variant == "sparse":
        bf16 = _bf16()
        dist_bf = np.ascontiguousarray(dist_t.astype(bf16))
        # fold (1 - p_gen) into alpha before the bf16 round
        alphaq = (alph_t * (1.0 - p_gen.transpose(0, 2, 1))).astype(bf16)
        n_tchunk = L_DEC // P
        in_maps = []
        for b in range(B):
            idx, ptrg = metas[b]
            # gather alpha rows by owning macro on the host; zero the
            # padding rows so they contribute nothing to the matmul
            alphag = alphaq[b][idx[:, :, 0]]          # [N_MACRO, P, L_DEC]
            alphag[ptrg[:, :, 0] < 0] = 0
            # [p, m*L_DEC + t] layout -> one DMA loads the whole arena
            agh = np.ascontiguousarray(
                alphag.transpose(1, 0, 2).reshape(P, N_MACRO_SPARSE * L_DEC)
            )
            # smalls[p, 0:2] = pgen per t-chunk, smalls[p, 2:10] = grouped
            # pointer values as f32, padded to 128 cols for line-rate DMA
            smalls = np.zeros((P, P), np.float32)
            smalls[:, :n_tchunk] = p_gen[b, :, 0].reshape(n_tchunk, P).T
            smalls[:, n_tchunk : n_tchunk + N_MACRO_SPARSE] = (
                ptrg[:, :, 0].T.astype(np.float32)
            )
            in_maps.append(
                {"dist": dist_bf[b], "smalls": smalls, "agh": agh}
            )
    else:
        dist_f32 = np.ascontiguousarray(dist_t)
        alph_f32 = np.ascontiguousarray(alph_t)
        in_maps = [
            {"dist": dist_f32[b], "pgen": p_gen[b], "alpha": alph_f32[b],
             "ptr": np.ascontiguousarray(ptr[b].reshape(L_SRC, 1))}
            for b in range(B)
        ]
    return variant, in_maps


def run(dist_t, p_gen, alph_t, batch_vocab, pointer, trace=False,
        force_variant=None, **spmd_kwargs):
    """Run the kernel; returns (output, BassKernelResults)."""
    assert batch_vocab.shape[0] == V_EXT
    variant, in_maps = _prep(dist_t, p_gen, alph_t, pointer)
    if force_variant == "dense" and variant == "sparse":
        ptr = np.asarray(pointer).astype(np.int32).reshape(B, L_SRC)
        dist_f32 = np.ascontiguousarray(np.asarray(dist_t, dtype=np.float32))
        alph_f32 = np.ascontiguousarray(np.asarray(alph_t, dtype=np.float32))
        p_gen_f = np.ascontiguousarray(
            np.asarray(p_gen, dtype=np.float32).reshape(B, L_DEC, 1)
        )
        in_maps = [
            {"dist": dist_f32[b], "pgen": p_gen_f[b], "alpha": alph_f32[b],
             "ptr": np.ascontiguousarray(ptr[b].reshape(L_SRC, 1))}
            for b in range(B)
        ]
        variant = "dense"
    run.last_variant = variant
    res = None
    for attempt in range(3):
        try:
            res = run_bass_kernel_spmd(
                _get_nc(variant), in_maps, list(range(NCORES)),
                trace=trace and attempt == 0, **spmd_kwargs
            )
            break
        except Exception:
            # transient device-state failures (e.g. NRT_EXEC_UNIT_UNRECOVERABLE
            # left over from a previous profiled session) sometimes clear on
            # retry; give it two more chances (untraced -- profiling itself
            # can be the destabilizer) before giving up
            if attempt == 2:
                raise
            import time

            time.sleep(2.0)
    outs = [res.results[b]["out"] for b in range(B)]
    out = np.stack([np.asarray(o, dtype=np.float32) for o in outs], axis=0)
    return out, res


def kernel(dist_t, p_gen, alph_t, batch_vocab, pointer):
    out, _ = run(dist_t, p_gen, alph_t, batch_vocab, pointer)
    return out
